# revision 10
# baseline (speedup 1.0000x reference)
"""Trainium2 Bass kernel for nn_Encoder (CNN stem + 2-layer ConvLSTM).

Self-contained: builds three Bass programs (compiled once per process) and
orchestrates four SPMD launches on 8 NeuronCores:

  A: CNN stem (3x conv+BN+ReLU+pool) + layer-0 input-gate conv Zx0,
     data-parallel over the 32 (b,t) images (4 per core).
  R: ConvLSTM recurrence (16 sequential steps; hidden-half gate conv +
     gate nonlinearities + state update), data-parallel over batch (B=2).
     Compiled once, launched twice (layer 0 and layer 1).
  C: layer-1 input-gate conv Zx1 over layer-0 hidden states,
     data-parallel over the 32 (b,t) items.

All matmuls run as float32r (TF32-like, ~1 PE cycle/row at free>=256).
Conv3x3 is 9 shifted matmuls accumulated in PSUM over zero-padded
flat buffers with a 1-element lead/tail pad; the pad columns of each
conv output span carry junk that is never read by valid outputs.
"""

import numpy as np

import concourse.bass as bass
import concourse.mybir as mybir
from concourse import bacc
from concourse.bass_utils import run_bass_kernel_spmd
from concourse.tile import TileContext

F32 = mybir.dt.float32
F32R = mybir.dt.float32r
AF = mybir.ActivationFunctionType
ALU = mybir.AluOpType

N_CORES = 8
B, T, H, W = 2, 16, 128, 128
HID = 256
EPS = 1e-5
IMGS = B * T           # 32
IPC = IMGS // N_CORES  # 4 images per core

# stem geometry: (rows, padded width, span length, lead-padded buffer len)
# level 0 input: 128x128 -> padded 130x130
P0 = 130
XPADN = P0 * P0 + 2          # host-side lead/tail padded flat image
# level 1 input: 64x64 -> padded 66
P1, S1 = 66, 64 * 66         # span covers 64 rows x 66 cols
Q1 = 66 * 66 + 2
# level 2 input: 32x32 -> padded 34
P2, S2 = 34, 32 * 34
Q2 = 34 * 34 + 2
# lstm spatial: 16x16 -> padded 18
PL, SL = 18, 16 * 18         # SL = 288
QL = 18 * 18 + 2             # 326

PROFILE = False
LAST_EXEC_NS = 0.0

_PROGRAMS = {}


def _taps():
    return [(ky, kx) for ky in range(3) for kx in range(3)]


def _ap(handle, offset, dims):
    return bass.AP(handle, offset, [list(d) for d in dims])


# --------------------------------------------------------------------------
# shared emitters
# --------------------------------------------------------------------------

def _emit_gate_conv(nc, psum_pool, wsb, src_slices, out_cb, n_ct=2):
    """z[ot] = sum_{tap,ct} W[tap,ct,:,ot*128:+128].T @ src[ct][tapoff:+288].

    wsb: SBUF tile [128, 9*n_ct*1024] (k=(tap*n_ct+ct) major, gate-out minor)
    src_slices: fn(ct, tapoff) -> rhs AP [128, 288]
    out_cb: fn(ot, psum_ap) emitted after the 18 matmuls of each out tile.
    """
    taps = _taps()
    for ot in range(8):
        ps = psum_pool.tile([128, SL], F32, tag="psg")
        n_k = len(taps) * n_ct
        k = 0
        for ti, (ky, kx) in enumerate(taps):
            for ct in range(n_ct):
                lhs = wsb[:, (ti * n_ct + ct) * 1024 + ot * 128:
                          (ti * n_ct + ct) * 1024 + ot * 128 + 128]
                nc.tensor.matmul(ps[:, :], lhs, src_slices(ct, ky * PL + kx),
                                 start=(k == 0), stop=(k == n_k - 1))
                k += 1
        out_cb(ot, ps)


def _load_gate_weights(nc, sb, w_dram, n_ct=2):
    """DMA [9, n_ct, 128, 1024] f32r weights into SBUF [128, 9*n_ct*1024]."""
    wsb = sb.tile([128, 9 * n_ct * 1024], F32R, tag="wgate")
    for ti in range(9):
        dst = wsb[:, ti * n_ct * 1024:(ti + 1) * n_ct * 1024].rearrange(
            "p (c o) -> p c o", c=n_ct)
        src = _ap(w_dram, ti * n_ct * 128 * 1024,
                  [[1024, 128], [128 * 1024, n_ct], [1, 1024]])
        nc.sync.dma_start(dst, src)
    return wsb


# --------------------------------------------------------------------------
# Launch A: CNN stem + Zx0
# --------------------------------------------------------------------------

def build_A():
    nc = bacc.Bacc("TRN2", target_bir_lowering=False, debug=False,
                   num_devices=N_CORES)
    xpad = nc.dram_tensor("xpad", [IPC, XPADN], F32R, kind="ExternalInput")
    w0t = nc.dram_tensor("w0t", [9, 64], F32R, kind="ExternalInput")
    b0t = nc.dram_tensor("b0t", [64, 1], F32, kind="ExternalInput")
    w1t = nc.dram_tensor("w1t", [9, 64, 128], F32R, kind="ExternalInput")
    b1t = nc.dram_tensor("b1t", [128, 1], F32, kind="ExternalInput")
    w2t = nc.dram_tensor("w2t", [9, 128, 256], F32R, kind="ExternalInput")
    b2t = nc.dram_tensor("b2t", [128, 2], F32, kind="ExternalInput")
    wx = nc.dram_tensor("wx", [9, 2, 128, 1024], F32R, kind="ExternalInput")
    lb = nc.dram_tensor("lb", [128, 8], F32, kind="ExternalInput")
    zx = nc.dram_tensor("zx", [IPC, 8, 128, SL], F32, kind="ExternalOutput")

    taps = _taps()
    with TileContext(nc) as tc:
        with (
            tc.tile_pool(name="wpool", bufs=1) as wp,
            tc.tile_pool(name="pads", bufs=1) as padp,
            tc.tile_pool(name="work", bufs=1) as wk,
            tc.tile_pool(name="psum", bufs=3, space="PSUM") as pp,
            tc.tile_pool(name="psumg", bufs=4, space="PSUM") as ppg,
            tc.tile_pool(name="psumd", bufs=1, space="PSUM") as ppd,
        ):
            w0sb = wp.tile([9, 64], F32R)
            nc.sync.dma_start(w0sb[:], w0t[:, :])
            w1sb = wp.tile([64, 9 * 128], F32R)
            nc.sync.dma_start(
                w1sb[:].rearrange("p (t o) -> p t o", t=9),
                _ap(w1t, 0, [[128, 64], [64 * 128, 9], [1, 128]]))
            w2sb = wp.tile([128, 9 * 256], F32R)
            nc.sync.dma_start(
                w2sb[:].rearrange("p (t o) -> p t o", t=9),
                _ap(w2t, 0, [[256, 128], [128 * 256, 9], [1, 256]]))
            b0sb = wp.tile([64, 1], F32)
            nc.sync.dma_start(b0sb[:], b0t[:, :])
            b1sb = wp.tile([128, 1], F32)
            nc.sync.dma_start(b1sb[:], b1t[:, :])
            b2sb = wp.tile([128, 2], F32)
            nc.sync.dma_start(b2sb[:], b2t[:, :])
            lbsb = wp.tile([128, 8], F32)
            nc.sync.dma_start(lbsb[:], lb[:, :])
            wxsb = _load_gate_weights(nc, wp, wx)

            x1p = padp.tile([64, Q1], F32R)
            x2p = padp.tile([128, Q2], F32R)
            spd = [padp.tile([128, QL], F32R, tag=f"spd{o}", name=f"spd{o}")
                   for o in range(2)]
            zsb = padp.tile([128, Q1], F32)
            nc.vector.memset(zsb[:], 0.0)
            nc.vector.tensor_copy(x1p[:], zsb[0:64, 0:Q1])
            nc.vector.tensor_copy(x2p[:], zsb[:, 0:Q2])
            nc.vector.tensor_copy(spd[0][:], zsb[:, 0:QL])
            nc.vector.tensor_copy(spd[1][:], zsb[:, 0:QL])

            CH0 = 16   # b0 output rows per chunk
            CH1 = 32   # b1 output rows per chunk
            for i in range(IPC):
                # ---- b0: conv 1->64 via im2col (contract 9) + pool ----
                for c in range(128 // CH0):
                    patch = wk.tile([9, CH0 * P0], F32R, tag="patch", bufs=2)
                    # patch[3ky+kx, s] = xpad[i][(130ky+kx) + r0*130 + s]
                    for ky in range(3):
                        psrc = _ap(xpad, i * XPADN + c * CH0 * P0 + P0 * ky,
                                   [[1, 3], [1, CH0 * P0]])
                        nc.sync.dma_start(patch[3 * ky:3 * ky + 3, :], psrc)
                    y0 = wk.tile([64, CH0 * P0], F32, tag="y0")
                    nsub = (CH0 * P0 + 511) // 512
                    for s in range(nsub):
                        lo = s * 512
                        ln = min(512, CH0 * P0 - lo)
                        ps = pp.tile([128, 512], F32, tag="pss")
                        nc.tensor.matmul(ps[0:64, :ln], w0sb[:],
                                         patch[:, lo:lo + ln],
                                         start=True, stop=True)
                        # relu(x + bias) on DVE: (psum add bias) max 0
                        nc.vector.scalar_tensor_tensor(
                            y0[:, lo:lo + ln], ps[0:64, :ln],
                            b0sb[:, 0:1], zsb[0:64, lo:lo + ln],
                            ALU.add, ALU.max)
                    # pool 2x2: span rows CH0 x 130, valid cols 1..128
                    y3 = y0[:].rearrange("p (r c) -> p r c", c=P0)
                    pa = wk.tile([64, CH0 * 64], F32, tag="pa")
                    pa3 = pa[:].rearrange("p (r c) -> p r c", c=64)
                    nc.vector.tensor_tensor(
                        pa3, y3[:, :, 1:129:2], y3[:, :, 2:130:2], ALU.max)
                    r0 = c * CH0 // 2
                    dst = _ap(x1p.tensor, x1p.offset + 68 + r0 * P1,
                              [[x1p.ap[0][0], 64], [P1, CH0 // 2],
                               [1, 64]])
                    nc.vector.tensor_tensor(
                        dst, pa3[:, 0:CH0:2, :], pa3[:, 1:CH0:2, :], ALU.max)
                    for d in range(6):
                        pd = ppd.tile([128, 288], F32, tag="psd", name="psd")
                        nc.tensor.matmul(pd[:, :], wxsb[:, 0:128],
                                         wxsb[:, 128:128 + 288],
                                         start=True, stop=True)

                # ---- b1: conv 64->128 + pool ----
                for c in range(64 // CH1):
                    base = c * CH1 * P1
                    y1 = wk.tile([128, CH1 * P1], F32, tag="y1", bufs=2)
                    nsub = (CH1 * P1 + 511) // 512
                    for s in range(nsub):
                        lo = s * 512
                        ln = min(512, CH1 * P1 - lo)
                        ps = pp.tile([128, 512], F32, tag="pss")
                        for ti, (ky, kx) in enumerate(taps):
                            off = ky * P1 + kx + base + lo
                            nc.tensor.matmul(
                                ps[:, :ln],
                                w1sb[:, ti * 128:(ti + 1) * 128],
                                x1p[:, off:off + ln],
                                start=(ti == 0), stop=(ti == 8))
                        nc.scalar.activation(y1[:, lo:lo + ln], ps[:, :ln],
                                             AF.Relu, bias=b1sb[:, 0:1])
                    y3 = y1[:].rearrange("p (r c) -> p r c", c=P1)
                    pa1 = wk.tile([128, CH1 * 32], F32, tag="pa1")
                    pa3 = pa1[:].rearrange("p (r c) -> p r c", c=32)
                    nc.vector.tensor_tensor(
                        pa3, y3[:, :, 1:65:2], y3[:, :, 2:66:2], ALU.max)
                    r0 = c * CH1 // 2
                    dst = _ap(x2p.tensor, x2p.offset + 36 + r0 * P2,
                              [[x2p.ap[0][0], 128], [P2, CH1 // 2],
                               [1, 32]])
                    nc.vector.tensor_tensor(
                        dst, pa3[:, 0:CH1:2, :], pa3[:, 1:CH1:2, :], ALU.max)

                # ---- b2: conv 128->256 + pool ----
                for o in range(2):
                    y2 = wk.tile([128, S2], F32, tag="y2")
                    nsub = (S2 + 511) // 512
                    for s in range(nsub):
                        lo = s * 512
                        ln = min(512, S2 - lo)
                        ps = pp.tile([128, 512], F32, tag="pss")
                        for ti, (ky, kx) in enumerate(taps):
                            off = ky * P2 + kx + lo
                            nc.tensor.matmul(
                                ps[:, :ln],
                                w2sb[:, ti * 256 + o * 128:
                                     ti * 256 + o * 128 + 128],
                                x2p[:, off:off + ln],
                                start=(ti == 0), stop=(ti == 8))
                        nc.scalar.activation(y2[:, lo:lo + ln], ps[:, :ln],
                                             AF.Relu, bias=b2sb[:, o:o + 1])
                    y3 = y2[:].rearrange("p (r c) -> p r c", c=P2)
                    pa2 = wk.tile([128, 32 * 16], F32, tag="pa2")
                    pa3 = pa2[:].rearrange("p (r c) -> p r c", c=16)
                    nc.vector.tensor_tensor(
                        pa3, y3[:, :, 1:33:2], y3[:, :, 2:34:2], ALU.max)
                    sp = spd[o]
                    dst = _ap(sp.tensor, sp.offset + 20,
                              [[sp.ap[0][0], 128], [PL, 16], [1, 16]])
                    nc.vector.tensor_tensor(
                        dst, pa3[:, 0:32:2, :], pa3[:, 1:32:2, :], ALU.max)

                # ---- Zx0 ----
                def out_cb(ot, ps, i=i):
                    zs = wk.tile([128, SL], F32, tag="zxs", bufs=2,
                                 name="zxs")
                    nc.scalar.activation(zs[:], ps[:, :], AF.Identity,
                                         bias=lbsb[:, ot:ot + 1])
                    nc.sync.dma_start(zx[i, ot], zs[:])
                _emit_gate_conv(
                    nc, ppg, wxsb,
                    lambda ct, toff: spd[ct][:, toff:toff + SL],
                    out_cb)
    nc.compile()
    return nc


# --------------------------------------------------------------------------
# Launch R: ConvLSTM recurrence (one layer, one batch element per core)
# --------------------------------------------------------------------------

def build_R():
    nc = bacc.Bacc("TRN2", target_bir_lowering=False, debug=False,
                   num_devices=N_CORES)
    zx = nc.dram_tensor("zx", [T, 8, 128, SL], F32, kind="ExternalInput")
    wh = nc.dram_tensor("wh", [9, 2, 128, 1024], F32R, kind="ExternalInput")
    hpad_seq = nc.dram_tensor("hpad_seq", [T, 2, 128, QL], F32R,
                              kind="ExternalOutput")
    hval_seq = nc.dram_tensor("hval_seq", [T, 2, 128, 256], F32,
                              kind="ExternalOutput")

    # gate channel blocks: i: ot 0-1, f: 2-3, o: 4-5, g: 6-7
    with TileContext(nc) as tc:
        with (
            tc.tile_pool(name="wpool", bufs=1) as wp,
            tc.tile_pool(name="state", bufs=1) as stp,
            tc.tile_pool(name="work", bufs=2) as wk,
            tc.tile_pool(name="psum", bufs=6, space="PSUM") as pp,
            tc.tile_pool(name="psumd", bufs=1, space="PSUM") as ppd,
        ):
            whsb = _load_gate_weights(nc, wp, wh)
            hp = [stp.tile([128, QL], F32R, tag=f"hp{c}", name=f"hp{c}")
                  for c in range(2)]
            cs = [stp.tile([128, SL], F32, tag=f"cs{c}", name=f"cs{c}")
                  for c in range(2)]
            zsb = stp.tile([128, QL], F32)
            nc.vector.memset(zsb[:], 0.0)
            for c in range(2):
                nc.vector.tensor_copy(hp[c][:], zsb[:, 0:QL])
                nc.vector.memset(cs[c][:], 0.0)

            taps = _taps()
            for t in range(T):
                zxsb = wk.tile([128, 8 * SL], F32, tag="zxsb")
                nc.sync.dma_start(
                    zxsb[:].rearrange("p (o n) -> p o n", o=8),
                    _ap(zx, t * 8 * 128 * SL,
                        [[SL, 128], [128 * SL, 8], [1, SL]]))

                ga = {}
                hss = {}
                for ot in (0, 2, 4, 6, 1, 3, 5, 7):
                    ps = pp.tile([128, SL], F32, tag="psr")
                    k = 0
                    for ti, (ky, kx) in enumerate(taps):
                        for ct in range(2):
                            lhs = whsb[:, (ti * 2 + ct) * 1024 + ot * 128:
                                       (ti * 2 + ct) * 1024 + ot * 128 + 128]
                            toff = ky * PL + kx
                            nc.tensor.matmul(ps[:, :], lhs,
                                             hp[ct][:, toff:toff + SL],
                                             start=(k == 0), stop=(k == 17))
                            k += 1
                    zt = wk.tile([128, SL], F32, tag=f"zt{ot}")
                    nc.vector.tensor_tensor(zt[:], ps[:, :],
                                            zxsb[:, ot * SL:(ot + 1) * SL],
                                            ALU.add)
                    g = wk.tile([128, SL], F32, tag=f"ga{ot}")
                    nc.scalar.activation(
                        g[:], zt[:], AF.Tanh if ot >= 6 else AF.Sigmoid)
                    ga[ot] = g

                    if ot in (6, 7):
                        ht = ot - 6
                        gi, gf, go, gg = (ga[ht], ga[2 + ht], ga[4 + ht],
                                          ga[6 + ht])
                        t1 = wk.tile([128, SL], F32, tag=f"t1{ht}",
                                     name=f"t1{ht}")
                        nc.vector.tensor_tensor(t1[:], gf[:], cs[ht][:],
                                                ALU.mult)
                        t2 = wk.tile([128, SL], F32, tag=f"t2{ht}",
                                     name=f"t2{ht}")
                        nc.vector.tensor_tensor(t2[:], gi[:], gg[:], ALU.mult)
                        nc.vector.tensor_tensor(cs[ht][:], t1[:], t2[:],
                                                ALU.add)
                        th = wk.tile([128, SL], F32, tag=f"th{ht}",
                                     name=f"th{ht}")
                        nc.scalar.activation(th[:], cs[ht][:], AF.Tanh)
                        hs = wk.tile([128, SL], F32, tag=f"hs{ht}",
                                     name=f"hs{ht}")
                        nc.vector.tensor_tensor(hs[:], go[:], th[:], ALU.mult)
                        hss[ht] = hs
                # HAM warmers: keep the PE clock at 8/8 through the serial
                # gate-chain gap. No deps (read only whsb), never read back.
                for d in range(28):
                    pd = ppd.tile([128, SL], F32, tag="psd", name="psd")
                    nc.tensor.matmul(pd[:, :], whsb[:, 0:128],
                                     whsb[:, 128:128 + SL],
                                     start=True, stop=True)
                # hp may only be overwritten after ALL 8 otile convs of this
                # step have read h_{t-1}; emit the copies after the loop.
                for ht in range(2):
                    hs = hss[ht]
                    hpt = hp[ht]
                    dst = _ap(hpt.tensor, hpt.offset + 20,
                              [[hpt.ap[0][0], 128], [PL, 16], [1, 16]])
                    src = _ap(hs.tensor, hs.offset + 1,
                              [[hs.ap[0][0], 128], [PL, 16], [1, 16]])
                    nc.vector.tensor_copy(dst, src)
                    nc.sync.dma_start(hpad_seq[t, ht], hpt[:])
                    nc.sync.dma_start(hval_seq[t, ht], src)
    nc.compile()
    return nc


# --------------------------------------------------------------------------
# Launch C: layer-1 input-gate conv over layer-0 hidden states
# --------------------------------------------------------------------------

def build_C():
    nc = bacc.Bacc("TRN2", target_bir_lowering=False, debug=False,
                   num_devices=N_CORES)
    hpad = nc.dram_tensor("hpad", [IPC, 2, 128, QL], F32R,
                          kind="ExternalInput")
    wx = nc.dram_tensor("wx", [9, 2, 128, 1024], F32R, kind="ExternalInput")
    lb = nc.dram_tensor("lb", [128, 8], F32, kind="ExternalInput")
    zx = nc.dram_tensor("zx", [IPC, 8, 128, SL], F32, kind="ExternalOutput")

    with TileContext(nc) as tc:
        with (
            tc.tile_pool(name="wpool", bufs=1) as wp,
            tc.tile_pool(name="work", bufs=2) as wk,
            tc.tile_pool(name="psum", bufs=4, space="PSUM") as pp,
        ):
            wxsb = _load_gate_weights(nc, wp, wx)
            lbsb = wp.tile([128, 8], F32)
            nc.sync.dma_start(lbsb[:], lb[:, :])
            for i in range(IPC):
                hsb = wk.tile([128, 2 * QL], F32R, tag="hsb")
                nc.sync.dma_start(
                    hsb[:].rearrange("p (c n) -> p c n", c=2),
                    _ap(hpad, i * 2 * 128 * QL,
                        [[QL, 128], [128 * QL, 2], [1, QL]]))

                def out_cb(ot, ps, i=i):
                    zs = wk.tile([128, SL], F32, tag="zxs")
                    nc.scalar.activation(zs[:], ps[:, :], AF.Identity,
                                         bias=lbsb[:, ot:ot + 1])
                    nc.sync.dma_start(zx[i, ot], zs[:])

                _emit_gate_conv(
                    nc, pp, wxsb,
                    lambda ct, toff, hsb=hsb: hsb[:, ct * QL + toff:
                                                  ct * QL + toff + SL],
                    out_cb)
    nc.compile()
    return nc


# --------------------------------------------------------------------------
# host orchestration
# --------------------------------------------------------------------------

def _get_programs():
    if not _PROGRAMS:
        _PROGRAMS["A"] = build_A()
        _PROGRAMS["R"] = build_R()
        _PROGRAMS["C"] = build_C()
    return _PROGRAMS


def _enable_profiling():
    import sys
    import types
    import concourse.bass_utils as bu
    bu.upload_artifacts = lambda tmpdir: tmpdir
    if "antenv.axon_hooks" in sys.modules:
        return
    try:
        from trn_agent_boot.trn_boot import _ntff_profile_via_ctypes
        hook = _ntff_profile_via_ctypes("/opt/axon/libaxon_pjrt.so")
        m = types.ModuleType("antenv.axon_hooks")
        m.get_axon_ntff_profile_hook = lambda: hook
        sys.modules["antenv.axon_hooks"] = m
    except Exception:
        pass


def _run(nc, in_maps):
    global LAST_EXEC_NS
    if PROFILE:
        _enable_profiling()
        res = run_bass_kernel_spmd(nc, in_maps, list(range(N_CORES)),
                                   trace=True)
        if res.exec_time_ns is not None:
            LAST_EXEC_NS += res.exec_time_ns
    else:
        res = run_bass_kernel_spmd(nc, in_maps, list(range(N_CORES)))
    return res.results


def _gate_weight_pack(lw, lo):
    """lw [1024, cin+256, 3, 3] -> (wx [9,2,128,1024], wh [9,2,128,1024])."""
    cin = lw.shape[1] - HID
    out = []
    for base in (0, cin):
        w = np.empty((9, 2, 128, 1024), np.float32)
        for ti, (ky, kx) in enumerate(_taps()):
            for ct in range(2):
                sl = lw[:, base + ct * 128: base + (ct + 1) * 128, ky, kx]
                w[ti, ct] = np.ascontiguousarray(sl.T)
        out.append(w)
    return out[0], out[1]


def kernel(x, w0, b0, g0, be0, m0, v0,
           w1, b1, g1, be1, m1, v1,
           w2, b2, g2, be2, m2, v2,
           lw0, lb0, lw1, lb1):
    global LAST_EXEC_NS
    LAST_EXEC_NS = 0.0
    progs = _get_programs()

    x = np.asarray(x, np.float32)
    nB, nT = x.shape[0], x.shape[1]
    assert (nB, nT) == (B, T)

    # ---- host prep: padded images ----
    imgs = x.reshape(IMGS, H, W)
    xpad = np.zeros((IMGS, XPADN), np.float32)
    v = xpad[:, 1:1 + P0 * P0].reshape(IMGS, P0, P0)
    v[:, 1:129, 1:129] = imgs

    # ---- stem weights with BN folding ----
    def fold(w, b, g, be, m, vv):
        scale = (np.asarray(g) / np.sqrt(np.asarray(vv) + EPS)).astype(np.float32)
        shift = (np.asarray(be) - np.asarray(m) * scale).astype(np.float32)
        bias = np.asarray(b) * scale + shift
        wt = np.asarray(w) * scale[:, None, None, None]
        return wt.astype(np.float32), bias.astype(np.float32)

    w0f, bias0 = fold(w0, b0, g0, be0, m0, v0)
    w1f, bias1 = fold(w1, b1, g1, be1, m1, v1)
    w2f, bias2 = fold(w2, b2, g2, be2, m2, v2)

    w0t = np.empty((9, 64), np.float32)
    w1t = np.empty((9, 64, 128), np.float32)
    w2t = np.empty((9, 128, 256), np.float32)
    for ti, (ky, kx) in enumerate(_taps()):
        w0t[ti] = w0f[:, 0, ky, kx]
        w1t[ti] = w1f[:, :, ky, kx].T
        w2t[ti] = w2f[:, :, ky, kx].T
    b0a = bias0.reshape(64, 1)
    b1a = bias1.reshape(128, 1)
    b2a = np.ascontiguousarray(bias2.reshape(2, 128).T)

    wx0, wh0 = _gate_weight_pack(np.asarray(lw0, np.float32), lb0)
    wx1, wh1 = _gate_weight_pack(np.asarray(lw1, np.float32), lb1)
    lb0a = np.ascontiguousarray(np.asarray(lb0, np.float32).reshape(8, 128).T)
    lb1a = np.ascontiguousarray(np.asarray(lb1, np.float32).reshape(8, 128).T)

    # ---- launch A ----
    in_a = [{"xpad": xpad[c * IPC:(c + 1) * IPC], "w0t": w0t, "b0t": b0a,
             "w1t": w1t, "b1t": b1a, "w2t": w2t, "b2t": b2a,
             "wx": wx0, "lb": lb0a} for c in range(N_CORES)]
    res = _run(progs["A"], in_a)
    zx0 = np.concatenate([res[c]["zx"] for c in range(N_CORES)], axis=0)
    # zx0: [32, 8, 128, SL], img index = b*T + t

    # ---- launch R (layer 0) ----
    in_r = [{"zx": zx0[(c % B) * T:(c % B) * T + T], "wh": wh0}
            for c in range(N_CORES)]
    res = _run(progs["R"], in_r)
    hpad0 = [res[b]["hpad_seq"] for b in range(B)]  # [T, 2, 128, QL] each

    # ---- launch C (Zx1) ----
    hp_items = np.concatenate(hpad0, axis=0)  # [32, 2, 128, QL], b-major
    in_c = [{"hpad": hp_items[c * IPC:(c + 1) * IPC], "wx": wx1, "lb": lb1a}
            for c in range(N_CORES)]
    res = _run(progs["C"], in_c)
    zx1 = np.concatenate([res[c]["zx"] for c in range(N_CORES)], axis=0)

    # ---- launch R (layer 1) ----
    in_r = [{"zx": zx1[(c % B) * T:(c % B) * T + T], "wh": wh1}
            for c in range(N_CORES)]
    res = _run(progs["R"], in_r)

    z_seq = np.empty((B, T, HID, 16, 16), np.float32)
    for b_ in range(B):
        hv = res[b_]["hval_seq"]  # [T, 2, 128, 256]
        z_seq[b_] = hv.reshape(T, HID, 16, 16)
    z_last = np.ascontiguousarray(z_seq[:, -1])
    return z_seq, z_last


# revision 13
# speedup vs baseline: 1.3989x; 1.3989x over previous
"""Trainium2 Bass kernel for nn_Encoder (CNN stem + 2-layer ConvLSTM).

Self-contained: builds three Bass programs (compiled once per process) and
orchestrates four SPMD launches on 8 NeuronCores:

  A: CNN stem (3x conv+BN+ReLU+pool) + layer-0 input-gate conv Zx0,
     data-parallel over the 32 (b,t) images (4 per core).
  R: ConvLSTM recurrence (16 sequential steps; hidden-half gate conv +
     gate nonlinearities + state update), data-parallel over batch (B=2).
     Compiled once, launched twice (layer 0 and layer 1).
  C: layer-1 input-gate conv Zx1 over layer-0 hidden states,
     data-parallel over the 32 (b,t) items.

All matmuls run as float32r (TF32-like, ~1 PE cycle/row at free>=256).
Conv3x3 is 9 shifted matmuls accumulated in PSUM over zero-padded
flat buffers with a 1-element lead/tail pad; the pad columns of each
conv output span carry junk that is never read by valid outputs.
"""

import numpy as np

import concourse.bass as bass
import concourse.mybir as mybir
from concourse import bacc
from concourse.bass_utils import run_bass_kernel_spmd
from concourse.tile import TileContext

F32 = mybir.dt.float32
F32R = mybir.dt.float32r
AF = mybir.ActivationFunctionType
ALU = mybir.AluOpType

N_CORES = 8
B, T, H, W = 2, 16, 128, 128
HID = 256
EPS = 1e-5
IMGS = B * T           # 32
IPC = IMGS // N_CORES  # 4 images per core

# stem geometry: (rows, padded width, span length, lead-padded buffer len)
# level 0 input: 128x128 -> padded 130x130
P0 = 130
XPADN = P0 * P0 + 2          # host-side lead/tail padded flat image
# level 1 input: 64x64 -> padded 66
P1, S1 = 66, 64 * 66         # span covers 64 rows x 66 cols
Q1 = 66 * 66 + 2
# level 2 input: 32x32 -> padded 34
P2, S2 = 34, 32 * 34
Q2 = 34 * 34 + 2
# lstm spatial: 16x16 -> padded 18
PL, SL = 18, 16 * 18         # SL = 288
QL = 18 * 18 + 2             # 326

PROFILE = False
LAST_EXEC_NS = 0.0

_PROGRAMS = {}


def _taps():
    return [(ky, kx) for ky in range(3) for kx in range(3)]


def _ap(handle, offset, dims):
    return bass.AP(handle, offset, [list(d) for d in dims])


# --------------------------------------------------------------------------
# shared emitters
# --------------------------------------------------------------------------

def _gate_lhs(wsb, ot, ti, ct, n_ct=2):
    base = ot * 9 * n_ct * 128 + (ti * n_ct + ct) * 128
    return wsb[:, base:base + 128]


def _emit_gate_conv(nc, psum_pool, wsb, src_slices, out_cb, n_ct=2,
                    order=tuple(range(8))):
    """z[ot] = sum_{tap,ct} W[tap,ct,:,ot*128:+128].T @ src[ct][tapoff:+288]."""
    taps = _taps()
    for ot in order:
        ps = psum_pool.tile([128, SL], F32, tag="psg")
        n_k = len(taps) * n_ct
        k = 0
        for ti, (ky, kx) in enumerate(taps):
            for ct in range(n_ct):
                nc.tensor.matmul(ps[:, :], _gate_lhs(wsb, ot, ti, ct, n_ct),
                                 src_slices(ct, ky * PL + kx),
                                 start=(k == 0), stop=(k == n_k - 1))
                k += 1
        out_cb(ot, ps)


def _load_gate_weights(nc, sb, w_dram, n_ct=2):
    """DMA [9, n_ct, 128, 1024] f32r weights into SBUF [128, 8*9*n_ct*128],
    otile-major so consumers of otile o only wait on slab o's DMA."""
    slab = 9 * n_ct * 128
    wsb = sb.tile([128, 8 * slab], F32R, tag="wgate")
    for o in range(8):
        for ct in range(n_ct):
            dst = _ap(wsb.tensor, wsb.offset + o * slab + ct * 128,
                      [[wsb.ap[0][0], 128], [n_ct * 128, 9], [1, 128]])
            src = _ap(w_dram, ct * 128 * 1024 + o * 128,
                      [[1024, 128], [n_ct * 128 * 1024, 9], [1, 128]])
            nc.sync.dma_start(dst, src)
    return wsb


# --------------------------------------------------------------------------
# Launch A: CNN stem + Zx0
# --------------------------------------------------------------------------

def build_A():
    nc = bacc.Bacc("TRN2", target_bir_lowering=False, debug=False,
                   num_devices=N_CORES)
    xpad = nc.dram_tensor("xpad", [IPC, XPADN], F32R, kind="ExternalInput")
    w0t = nc.dram_tensor("w0t", [128, 64], F32R, kind="ExternalInput")
    b0t = nc.dram_tensor("b0t", [64, 1], F32, kind="ExternalInput")
    w1t = nc.dram_tensor("w1t", [9, 128, 128], F32R, kind="ExternalInput")
    b1t = nc.dram_tensor("b1t", [128, 1], F32, kind="ExternalInput")
    w2t = nc.dram_tensor("w2t", [9, 128, 256], F32R, kind="ExternalInput")
    b2t = nc.dram_tensor("b2t", [128, 2], F32, kind="ExternalInput")
    wx = nc.dram_tensor("wx", [9, 2, 128, 1024], F32R, kind="ExternalInput")
    lb = nc.dram_tensor("lb", [128, 8], F32, kind="ExternalInput")
    zx = nc.dram_tensor("zx", [IPC, 8, 128, SL], F32, kind="ExternalOutput")

    taps = _taps()
    with TileContext(nc) as tc:
        with (
            tc.tile_pool(name="wpool", bufs=1) as wp,
            tc.tile_pool(name="pads", bufs=1) as padp,
            tc.tile_pool(name="work", bufs=1) as wk,
            tc.tile_pool(name="psum", bufs=3, space="PSUM") as pp,
            tc.tile_pool(name="psumg", bufs=5, space="PSUM") as ppg,
        ):
            w0sb = wp.tile([128, 64], F32R)
            nc.sync.dma_start(w0sb[:], w0t[:, :])
            w1sb = wp.tile([128, 9 * 128], F32R)
            nc.sync.dma_start(
                w1sb[:].rearrange("p (t o) -> p t o", t=9),
                _ap(w1t, 0, [[128, 128], [128 * 128, 9], [1, 128]]))
            w2sb = wp.tile([128, 9 * 256], F32R)
            nc.sync.dma_start(
                w2sb[:].rearrange("p (t o) -> p t o", t=9),
                _ap(w2t, 0, [[256, 128], [128 * 256, 9], [1, 256]]))
            b0sb = wp.tile([64, 1], F32)
            nc.sync.dma_start(b0sb[:], b0t[:, :])
            b1sb = wp.tile([128, 1], F32)
            nc.sync.dma_start(b1sb[:], b1t[:, :])
            b2sb = wp.tile([128, 2], F32)
            nc.sync.dma_start(b2sb[:], b2t[:, :])
            lbsb = wp.tile([128, 8], F32)
            nc.sync.dma_start(lbsb[:], lb[:, :])
            wxsb = _load_gate_weights(nc, wp, wx)

            x1p = padp.tile([128, Q1], F32R)
            x2p = padp.tile([128, Q2], F32R)
            spd = [padp.tile([128, QL], F32R, tag=f"spd{o}", name=f"spd{o}")
                   for o in range(2)]
            zsb = padp.tile([128, Q1], F32)
            nc.vector.memset(zsb[:], 0.0)
            nc.vector.tensor_copy(x1p[:], zsb[:, 0:Q1])
            nc.vector.tensor_copy(x2p[:], zsb[:, 0:Q2])
            nc.vector.tensor_copy(spd[0][:], zsb[:, 0:QL])
            nc.vector.tensor_copy(spd[1][:], zsb[:, 0:QL])

            CH0 = 16   # b0 output rows per chunk
            CH1 = 32   # b1 output rows per chunk
            pch = [wk.tile([128, CH0 * P0], F32R, tag=f"patch{j}",
                           name=f"patch{j}") for j in range(2)]
            nc.vector.tensor_copy(pch[0][:, :], zsb[:, 0:CH0 * P0])
            nc.vector.tensor_copy(pch[1][:, :], zsb[:, 0:CH0 * P0])
            for i in range(IPC):
                # ---- b0: conv 1->64 via im2col (contract 9 pad 128) ----
                for c in range(128 // CH0):
                    patch = pch[c % 2]
                    # patch[3ky+kx, s] = xpad[i][(130ky+kx) + r0*130 + s]
                    for ky in range(3):
                        psrc = _ap(xpad, i * XPADN + c * CH0 * P0 + P0 * ky,
                                   [[1, 3], [1, CH0 * P0]])
                        nc.sync.dma_start(patch[3 * ky:3 * ky + 3, :], psrc)
                    y0 = wk.tile([64, CH0 * P0], F32, tag="y0")
                    nsub = (CH0 * P0 + 511) // 512
                    for s in range(nsub):
                        lo = s * 512
                        ln = min(512, CH0 * P0 - lo)
                        ps = pp.tile([128, 512], F32, tag="pss")
                        nc.tensor.matmul(ps[0:64, :ln], w0sb[:],
                                         patch[:, lo:lo + ln],
                                         start=True, stop=True)
                        nc.scalar.activation(y0[:, lo:lo + ln],
                                             ps[0:64, :ln],
                                             AF.Relu, bias=b0sb[:, 0:1])
                    # pool 2x2: span rows CH0 x 130, valid cols 1..128
                    y3 = y0[:].rearrange("p (r c) -> p r c", c=P0)
                    pa = wk.tile([64, CH0 * 64], F32, tag="pa")
                    pa3 = pa[:].rearrange("p (r c) -> p r c", c=64)
                    nc.vector.tensor_tensor(
                        pa3, y3[:, :, 1:129:2], y3[:, :, 2:130:2], ALU.max)
                    r0 = c * CH0 // 2
                    dst = _ap(x1p.tensor, x1p.offset + 68 + r0 * P1,
                              [[x1p.ap[0][0], 64], [P1, CH0 // 2],
                               [1, 64]])
                    nc.vector.tensor_tensor(
                        dst, pa3[:, 0:CH0:2, :], pa3[:, 1:CH0:2, :], ALU.max)

                # ---- b1: conv 64->128 + pool ----
                for c in range(64 // CH1):
                    base = c * CH1 * P1
                    y1 = wk.tile([128, CH1 * P1], F32, tag="y1", bufs=2)
                    nsub = (CH1 * P1 + 511) // 512
                    for s in range(nsub):
                        lo = s * 512
                        ln = min(512, CH1 * P1 - lo)
                        ps = pp.tile([128, 512], F32, tag="pss")
                        for ti, (ky, kx) in enumerate(taps):
                            off = ky * P1 + kx + base + lo
                            nc.tensor.matmul(
                                ps[:, :ln],
                                w1sb[:, ti * 128:(ti + 1) * 128],
                                x1p[:, off:off + ln],
                                start=(ti == 0), stop=(ti == 8))
                        nc.scalar.activation(y1[:, lo:lo + ln], ps[:, :ln],
                                             AF.Relu, bias=b1sb[:, 0:1])
                    y3 = y1[:].rearrange("p (r c) -> p r c", c=P1)
                    pa1 = wk.tile([128, CH1 * 32], F32, tag="pa1")
                    pa3 = pa1[:].rearrange("p (r c) -> p r c", c=32)
                    nc.vector.tensor_tensor(
                        pa3, y3[:, :, 1:65:2], y3[:, :, 2:66:2], ALU.max)
                    r0 = c * CH1 // 2
                    dst = _ap(x2p.tensor, x2p.offset + 36 + r0 * P2,
                              [[x2p.ap[0][0], 128], [P2, CH1 // 2],
                               [1, 32]])
                    nc.vector.tensor_tensor(
                        dst, pa3[:, 0:CH1:2, :], pa3[:, 1:CH1:2, :], ALU.max)

                # ---- b2: conv 128->256 + pool ----
                for o in range(2):
                    y2 = wk.tile([128, S2], F32, tag="y2")
                    nsub = (S2 + 511) // 512
                    for s in range(nsub):
                        lo = s * 512
                        ln = min(512, S2 - lo)
                        ps = pp.tile([128, 512], F32, tag="pss")
                        for ti, (ky, kx) in enumerate(taps):
                            off = ky * P2 + kx + lo
                            nc.tensor.matmul(
                                ps[:, :ln],
                                w2sb[:, ti * 256 + o * 128:
                                     ti * 256 + o * 128 + 128],
                                x2p[:, off:off + ln],
                                start=(ti == 0), stop=(ti == 8))
                        nc.scalar.activation(y2[:, lo:lo + ln], ps[:, :ln],
                                             AF.Relu, bias=b2sb[:, o:o + 1])
                    y3 = y2[:].rearrange("p (r c) -> p r c", c=P2)
                    pa2 = wk.tile([128, 32 * 16], F32, tag="pa2")
                    pa3 = pa2[:].rearrange("p (r c) -> p r c", c=16)
                    nc.vector.tensor_tensor(
                        pa3, y3[:, :, 1:33:2], y3[:, :, 2:34:2], ALU.max)
                    sp = spd[o]
                    dst = _ap(sp.tensor, sp.offset + 20,
                              [[sp.ap[0][0], 128], [PL, 16], [1, 16]])
                    nc.vector.tensor_tensor(
                        dst, pa3[:, 0:32:2, :], pa3[:, 1:32:2, :], ALU.max)

                # ---- Zx0 ----
                def out_cb(ot, ps, i=i):
                    zs = wk.tile([128, SL], F32, tag="zxs", bufs=2,
                                 name="zxs")
                    nc.scalar.activation(zs[:], ps[:, :], AF.Identity,
                                         bias=lbsb[:, ot:ot + 1])
                    nc.sync.dma_start(zx[i, ot], zs[:])
                _emit_gate_conv(
                    nc, ppg, wxsb,
                    lambda ct, toff: spd[ct][:, toff:toff + SL],
                    out_cb)
    nc.compile()
    return nc


# --------------------------------------------------------------------------
# Launch R: ConvLSTM recurrence (one layer, one batch element per core)
# --------------------------------------------------------------------------

def build_R():
    nc = bacc.Bacc("TRN2", target_bir_lowering=False, debug=False,
                   num_devices=N_CORES)
    zx = nc.dram_tensor("zx", [T, 8, 128, SL], F32, kind="ExternalInput")
    wh = nc.dram_tensor("wh", [9, 2, 128, 1024], F32R, kind="ExternalInput")
    hpad_seq = nc.dram_tensor("hpad_seq", [T, 2, 128, QL], F32R,
                              kind="ExternalOutput")
    hval_seq = nc.dram_tensor("hval_seq", [T, 2, 128, 256], F32,
                              kind="ExternalOutput")

    # gate channel blocks: i: ot 0-1, f: 2-3, o: 4-5, g: 6-7
    with TileContext(nc) as tc:
        with (
            tc.tile_pool(name="wpool", bufs=1) as wp,
            tc.tile_pool(name="state", bufs=1) as stp,
            tc.tile_pool(name="work", bufs=2) as wk,
            tc.tile_pool(name="psum", bufs=6, space="PSUM") as pp,
        ):
            whsb = _load_gate_weights(nc, wp, wh)
            hp = [stp.tile([128, QL], F32R, tag=f"hp{c}", name=f"hp{c}")
                  for c in range(2)]
            cs = [stp.tile([128, SL], F32, tag=f"cs{c}", name=f"cs{c}")
                  for c in range(2)]
            zsb = stp.tile([128, QL], F32)
            nc.vector.memset(zsb[:], 0.0)
            for c in range(2):
                nc.vector.tensor_copy(hp[c][:], zsb[:, 0:QL])
                nc.vector.memset(cs[c][:], 0.0)

            taps = _taps()
            for t in range(T):
                zxsb = wk.tile([128, 8 * SL], F32, tag="zxsb")
                nc.sync.dma_start(
                    zxsb[:].rearrange("p (o n) -> p o n", o=8),
                    _ap(zx, t * 8 * 128 * SL,
                        [[SL, 128], [128 * SL, 8], [1, SL]]))

                # otile order: g gates (6,7) first, then f (2,3), i (0,1),
                # o last (4,5) -- lets the c-update run under the remaining
                # convs so the serial tail is only sigma(o), h, and the copy.
                ga = {}
                hss = {}
                for ot in (6, 7, 2, 3, 0, 1, 4, 5):
                    ps = pp.tile([128, SL], F32, tag="psr")
                    k = 0
                    for ti, (ky, kx) in enumerate(taps):
                        for ct in range(2):
                            toff = ky * PL + kx
                            nc.tensor.matmul(ps[:, :],
                                             _gate_lhs(whsb, ot, ti, ct),
                                             hp[ct][:, toff:toff + SL],
                                             start=(k == 0), stop=(k == 17))
                            k += 1
                    zt = wk.tile([128, SL], F32, tag=f"zt{ot}", name=f"zt{ot}")
                    nc.vector.tensor_tensor(zt[:], ps[:, :],
                                            zxsb[:, ot * SL:(ot + 1) * SL],
                                            ALU.add)
                    g = wk.tile([128, SL], F32, tag=f"ga{ot}", name=f"ga{ot}")
                    nc.scalar.activation(
                        g[:], zt[:], AF.Tanh if ot >= 6 else AF.Sigmoid)
                    ga[ot] = g

                    if ot in (0, 1):
                        # i gate done; g, f already done: c = f*c + i*g
                        ht = ot
                        t1 = wk.tile([128, SL], F32, tag=f"t1{ht}",
                                     name=f"t1{ht}")
                        nc.vector.tensor_tensor(t1[:], ga[2 + ht][:],
                                                cs[ht][:], ALU.mult)
                        t2 = wk.tile([128, SL], F32, tag=f"t2{ht}",
                                     name=f"t2{ht}")
                        nc.vector.tensor_tensor(t2[:], ga[ht][:],
                                                ga[6 + ht][:], ALU.mult)
                        nc.vector.tensor_tensor(cs[ht][:], t1[:], t2[:],
                                                ALU.add)
                        th = wk.tile([128, SL], F32, tag=f"th{ht}",
                                     name=f"th{ht}")
                        nc.scalar.activation(th[:], cs[ht][:], AF.Tanh)
                        hss[ht] = th
                    if ot in (4, 5):
                        ht = ot - 4
                        hs = wk.tile([128, SL], F32, tag=f"hs{ht}",
                                     name=f"hs{ht}")
                        nc.vector.tensor_tensor(hs[:], ga[ot][:],
                                                hss[ht][:], ALU.mult)
                        hss[ht] = hs
                # hp may only be overwritten after ALL 8 otile convs of this
                # step have read h_{t-1}; emit the copies after the loop.
                for ht in range(2):
                    hs = hss[ht]
                    hpt = hp[ht]
                    dst = _ap(hpt.tensor, hpt.offset + 20,
                              [[hpt.ap[0][0], 128], [PL, 16], [1, 16]])
                    src = _ap(hs.tensor, hs.offset + 1,
                              [[hs.ap[0][0], 128], [PL, 16], [1, 16]])
                    nc.vector.tensor_copy(dst, src)
                    nc.sync.dma_start(hpad_seq[t, ht], hpt[:])
                    nc.sync.dma_start(hval_seq[t, ht], src)
    nc.compile()
    return nc


# --------------------------------------------------------------------------
# Launch C: layer-1 input-gate conv over layer-0 hidden states
# --------------------------------------------------------------------------

def build_C():
    nc = bacc.Bacc("TRN2", target_bir_lowering=False, debug=False,
                   num_devices=N_CORES)
    hpad = nc.dram_tensor("hpad", [IPC, 2, 128, QL], F32R,
                          kind="ExternalInput")
    wx = nc.dram_tensor("wx", [9, 2, 128, 1024], F32R, kind="ExternalInput")
    lb = nc.dram_tensor("lb", [128, 8], F32, kind="ExternalInput")
    zx = nc.dram_tensor("zx", [IPC, 8, 128, SL], F32, kind="ExternalOutput")

    with TileContext(nc) as tc:
        with (
            tc.tile_pool(name="wpool", bufs=1) as wp,
            tc.tile_pool(name="work", bufs=2) as wk,
            tc.tile_pool(name="psum", bufs=4, space="PSUM") as pp,
        ):
            wxsb = _load_gate_weights(nc, wp, wx)
            lbsb = wp.tile([128, 8], F32)
            nc.sync.dma_start(lbsb[:], lb[:, :])
            for i in range(IPC):
                hsb = wk.tile([128, 2 * QL], F32R, tag="hsb")
                nc.sync.dma_start(
                    hsb[:].rearrange("p (c n) -> p c n", c=2),
                    _ap(hpad, i * 2 * 128 * QL,
                        [[QL, 128], [128 * QL, 2], [1, QL]]))

                def out_cb(ot, ps, i=i):
                    zs = wk.tile([128, SL], F32, tag="zxs")
                    nc.scalar.activation(zs[:], ps[:, :], AF.Identity,
                                         bias=lbsb[:, ot:ot + 1])
                    nc.sync.dma_start(zx[i, ot], zs[:])

                _emit_gate_conv(
                    nc, pp, wxsb,
                    lambda ct, toff, hsb=hsb: hsb[:, ct * QL + toff:
                                                  ct * QL + toff + SL],
                    out_cb)
    nc.compile()
    return nc


# --------------------------------------------------------------------------
# host orchestration
# --------------------------------------------------------------------------

def _get_programs():
    if not _PROGRAMS:
        _PROGRAMS["A"] = build_A()
        _PROGRAMS["R"] = build_R()
        _PROGRAMS["C"] = build_C()
    return _PROGRAMS


def _enable_profiling():
    import sys
    import types
    import concourse.bass_utils as bu
    bu.upload_artifacts = lambda tmpdir: tmpdir
    if "antenv.axon_hooks" in sys.modules:
        return
    try:
        from trn_agent_boot.trn_boot import _ntff_profile_via_ctypes
        hook = _ntff_profile_via_ctypes("/opt/axon/libaxon_pjrt.so")
        m = types.ModuleType("antenv.axon_hooks")
        m.get_axon_ntff_profile_hook = lambda: hook
        sys.modules["antenv.axon_hooks"] = m
    except Exception:
        pass


def _run(nc, in_maps):
    global LAST_EXEC_NS
    if PROFILE:
        _enable_profiling()
        res = run_bass_kernel_spmd(nc, in_maps, list(range(N_CORES)),
                                   trace=True)
        if res.exec_time_ns is not None:
            LAST_EXEC_NS += res.exec_time_ns
    else:
        res = run_bass_kernel_spmd(nc, in_maps, list(range(N_CORES)))
    return res.results


def _gate_weight_pack(lw, lo):
    """lw [1024, cin+256, 3, 3] -> (wx [9,2,128,1024], wh [9,2,128,1024])."""
    cin = lw.shape[1] - HID
    out = []
    for base in (0, cin):
        w = np.empty((9, 2, 128, 1024), np.float32)
        for ti, (ky, kx) in enumerate(_taps()):
            for ct in range(2):
                sl = lw[:, base + ct * 128: base + (ct + 1) * 128, ky, kx]
                w[ti, ct] = np.ascontiguousarray(sl.T)
        out.append(w)
    return out[0], out[1]


def kernel(x, w0, b0, g0, be0, m0, v0,
           w1, b1, g1, be1, m1, v1,
           w2, b2, g2, be2, m2, v2,
           lw0, lb0, lw1, lb1):
    global LAST_EXEC_NS
    LAST_EXEC_NS = 0.0
    progs = _get_programs()

    x = np.asarray(x, np.float32)
    nB, nT = x.shape[0], x.shape[1]
    assert (nB, nT) == (B, T)

    # ---- host prep: padded images ----
    imgs = x.reshape(IMGS, H, W)
    xpad = np.zeros((IMGS, XPADN), np.float32)
    v = xpad[:, 1:1 + P0 * P0].reshape(IMGS, P0, P0)
    v[:, 1:129, 1:129] = imgs

    # ---- stem weights with BN folding ----
    def fold(w, b, g, be, m, vv):
        scale = (np.asarray(g) / np.sqrt(np.asarray(vv) + EPS)).astype(np.float32)
        shift = (np.asarray(be) - np.asarray(m) * scale).astype(np.float32)
        bias = np.asarray(b) * scale + shift
        wt = np.asarray(w) * scale[:, None, None, None]
        return wt.astype(np.float32), bias.astype(np.float32)

    w0f, bias0 = fold(w0, b0, g0, be0, m0, v0)
    w1f, bias1 = fold(w1, b1, g1, be1, m1, v1)
    w2f, bias2 = fold(w2, b2, g2, be2, m2, v2)

    w0t = np.zeros((128, 64), np.float32)
    w1t = np.zeros((9, 128, 128), np.float32)
    w2t = np.empty((9, 128, 256), np.float32)
    for ti, (ky, kx) in enumerate(_taps()):
        w0t[ti] = w0f[:, 0, ky, kx]
        w1t[ti, 0:64] = w1f[:, :, ky, kx].T
        w2t[ti] = w2f[:, :, ky, kx].T
    b0a = bias0.reshape(64, 1)
    b1a = bias1.reshape(128, 1)
    b2a = np.ascontiguousarray(bias2.reshape(2, 128).T)

    wx0, wh0 = _gate_weight_pack(np.asarray(lw0, np.float32), lb0)
    wx1, wh1 = _gate_weight_pack(np.asarray(lw1, np.float32), lb1)
    lb0a = np.ascontiguousarray(np.asarray(lb0, np.float32).reshape(8, 128).T)
    lb1a = np.ascontiguousarray(np.asarray(lb1, np.float32).reshape(8, 128).T)

    # ---- launch A ----
    in_a = [{"xpad": xpad[c * IPC:(c + 1) * IPC], "w0t": w0t, "b0t": b0a,
             "w1t": w1t, "b1t": b1a, "w2t": w2t, "b2t": b2a,
             "wx": wx0, "lb": lb0a} for c in range(N_CORES)]
    res = _run(progs["A"], in_a)
    zx0 = np.concatenate([res[c]["zx"] for c in range(N_CORES)], axis=0)
    # zx0: [32, 8, 128, SL], img index = b*T + t

    # ---- launch R (layer 0) ----
    in_r = [{"zx": zx0[(c % B) * T:(c % B) * T + T], "wh": wh0}
            for c in range(N_CORES)]
    res = _run(progs["R"], in_r)
    hpad0 = [res[b]["hpad_seq"] for b in range(B)]  # [T, 2, 128, QL] each

    # ---- launch C (Zx1) ----
    hp_items = np.concatenate(hpad0, axis=0)  # [32, 2, 128, QL], b-major
    in_c = [{"hpad": hp_items[c * IPC:(c + 1) * IPC], "wx": wx1, "lb": lb1a}
            for c in range(N_CORES)]
    res = _run(progs["C"], in_c)
    zx1 = np.concatenate([res[c]["zx"] for c in range(N_CORES)], axis=0)

    # ---- launch R (layer 1) ----
    in_r = [{"zx": zx1[(c % B) * T:(c % B) * T + T], "wh": wh1}
            for c in range(N_CORES)]
    res = _run(progs["R"], in_r)

    z_seq = np.empty((B, T, HID, 16, 16), np.float32)
    for b_ in range(B):
        hv = res[b_]["hval_seq"]  # [T, 2, 128, 256]
        z_seq[b_] = hv.reshape(T, HID, 16, 16)
    z_last = np.ascontiguousarray(z_seq[:, -1])
    return z_seq, z_last


# revision 15
# speedup vs baseline: 1.5045x; 1.0755x over previous
"""Trainium2 Bass kernel for nn_Encoder (CNN stem + 2-layer ConvLSTM).

Self-contained: builds three Bass programs (compiled once per process) and
orchestrates four SPMD launches on 8 NeuronCores:

  A: CNN stem (3x conv+BN+ReLU+pool) + layer-0 input-gate conv Zx0,
     data-parallel over the 32 (b,t) images (4 per core).
  R: ConvLSTM recurrence (16 sequential steps; hidden-half gate conv +
     gate nonlinearities + state update), data-parallel over batch (B=2).
     Compiled once, launched twice (layer 0 and layer 1).
  C: layer-1 input-gate conv Zx1 over layer-0 hidden states,
     data-parallel over the 32 (b,t) items.

All matmuls run as float32r (TF32-like, ~1 PE cycle/row at free>=256).
Conv3x3 is 9 shifted matmuls accumulated in PSUM over zero-padded
flat buffers with a 1-element lead/tail pad; the pad columns of each
conv output span carry junk that is never read by valid outputs.
"""

import numpy as np

import concourse.bass as bass
import concourse.mybir as mybir
from concourse import bacc
from concourse.bass_utils import run_bass_kernel_spmd
from concourse.tile import TileContext

F32 = mybir.dt.float32
F32R = mybir.dt.float32r
AF = mybir.ActivationFunctionType
ALU = mybir.AluOpType

N_CORES = 8
B, T, H, W = 2, 16, 128, 128
HID = 256
EPS = 1e-5
IMGS = B * T           # 32
IPC = IMGS // N_CORES  # 4 images per core

# stem geometry: (rows, padded width, span length, lead-padded buffer len)
# level 0 input: 128x128 -> padded 130x130
P0 = 130
XPADN = P0 * P0 + 2          # host-side lead/tail padded flat image
# level 1 input: 64x64 -> padded 66
P1, S1 = 66, 64 * 66         # span covers 64 rows x 66 cols
Q1 = 66 * 66 + 2
# level 2 input: 32x32 -> padded 34
P2, S2 = 34, 32 * 34
Q2 = 34 * 34 + 2
# lstm spatial: 16x16 -> padded 18
PL, SL = 18, 16 * 18         # SL = 288
QL = 18 * 18 + 2             # 326

PROFILE = False
LAST_EXEC_NS = 0.0

_PROGRAMS = {}


def _taps():
    return [(ky, kx) for ky in range(3) for kx in range(3)]


def _ap(handle, offset, dims):
    return bass.AP(handle, offset, [list(d) for d in dims])


# --------------------------------------------------------------------------
# shared emitters
# --------------------------------------------------------------------------

def _gate_lhs(wsb, ot, ti, ct, n_ct=2):
    base = ot * 9 * n_ct * 128 + (ti * n_ct + ct) * 128
    return wsb[:, base:base + 128]


def _emit_gate_conv(nc, psum_pool, wsb, src_slices, out_cb, n_ct=2,
                    order=tuple(range(8))):
    """z[ot] = sum_{tap,ct} W[tap,ct,:,ot*128:+128].T @ src[ct][tapoff:+288]."""
    taps = _taps()
    for ot in order:
        ps = psum_pool.tile([128, SL], F32, tag="psg")
        n_k = len(taps) * n_ct
        k = 0
        for ti, (ky, kx) in enumerate(taps):
            for ct in range(n_ct):
                nc.tensor.matmul(ps[:, :], _gate_lhs(wsb, ot, ti, ct, n_ct),
                                 src_slices(ct, ky * PL + kx),
                                 start=(k == 0), stop=(k == n_k - 1))
                k += 1
        out_cb(ot, ps)


def _load_gate_weights(nc, sb, w_dram, n_ct=2, order=tuple(range(8))):
    """DMA host-prepacked [128, 8*9*n_ct*128] f32r weights (otile-major,
    contiguous slabs) into SBUF, on the SWDGE queues in use-order so the
    first consumer only waits on its own slab."""
    slab = 9 * n_ct * 128
    wsb = sb.tile([128, 8 * slab], F32R, tag="wgate")
    for o in order:
        nc.gpsimd.dma_start(wsb[:, o * slab:(o + 1) * slab],
                            w_dram[:, o * slab:(o + 1) * slab])
    return wsb


# --------------------------------------------------------------------------
# Launch A: CNN stem + Zx0
# --------------------------------------------------------------------------

def build_A():
    nc = bacc.Bacc("TRN2", target_bir_lowering=False, debug=False,
                   num_devices=N_CORES)
    xpad = nc.dram_tensor("xpad", [IPC, XPADN], F32R, kind="ExternalInput")
    w0t = nc.dram_tensor("w0t", [128, 64], F32R, kind="ExternalInput")
    b0t = nc.dram_tensor("b0t", [64, 1], F32, kind="ExternalInput")
    w1t = nc.dram_tensor("w1t", [9, 128, 128], F32R, kind="ExternalInput")
    b1t = nc.dram_tensor("b1t", [128, 1], F32, kind="ExternalInput")
    w2t = nc.dram_tensor("w2t", [9, 128, 256], F32R, kind="ExternalInput")
    b2t = nc.dram_tensor("b2t", [128, 2], F32, kind="ExternalInput")
    wx = nc.dram_tensor("wx", [128, 8 * 2304], F32R, kind="ExternalInput")
    lb = nc.dram_tensor("lb", [128, 8], F32, kind="ExternalInput")
    zx = nc.dram_tensor("zx", [IPC, 8, 128, SL], F32, kind="ExternalOutput")

    taps = _taps()
    with TileContext(nc) as tc:
        with (
            tc.tile_pool(name="wpool", bufs=1) as wp,
            tc.tile_pool(name="pads", bufs=1) as padp,
            tc.tile_pool(name="work", bufs=1) as wk,
            tc.tile_pool(name="psum", bufs=3, space="PSUM") as pp,
            tc.tile_pool(name="psumg", bufs=5, space="PSUM") as ppg,
        ):
            w0sb = wp.tile([128, 64], F32R)
            nc.sync.dma_start(w0sb[:], w0t[:, :])
            w1sb = wp.tile([128, 9 * 128], F32R)
            nc.gpsimd.dma_start(
                w1sb[:].rearrange("p (t o) -> p t o", t=9),
                _ap(w1t, 0, [[128, 128], [128 * 128, 9], [1, 128]]))
            w2sb = wp.tile([128, 9 * 256], F32R)
            nc.gpsimd.dma_start(
                w2sb[:].rearrange("p (t o) -> p t o", t=9),
                _ap(w2t, 0, [[256, 128], [128 * 256, 9], [1, 256]]))
            b0sb = wp.tile([64, 1], F32)
            nc.sync.dma_start(b0sb[:], b0t[:, :])
            b1sb = wp.tile([128, 1], F32)
            nc.sync.dma_start(b1sb[:], b1t[:, :])
            b2sb = wp.tile([128, 2], F32)
            nc.sync.dma_start(b2sb[:], b2t[:, :])
            lbsb = wp.tile([128, 8], F32)
            nc.sync.dma_start(lbsb[:], lb[:, :])
            wxsb = _load_gate_weights(nc, wp, wx)

            x1p = padp.tile([128, Q1], F32R)
            x2p = padp.tile([128, Q2], F32R)
            spd = [[padp.tile([128, QL], F32R, tag=f"spd{o}{par}",
                              name=f"spd{o}{par}") for o in range(2)]
                   for par in range(2)]
            zsb = padp.tile([128, Q1], F32)
            nc.vector.memset(zsb[:], 0.0)
            nc.vector.tensor_copy(x1p[:], zsb[:, 0:Q1])
            nc.vector.tensor_copy(x2p[:], zsb[:, 0:Q2])
            for par in range(2):
                for o in range(2):
                    nc.vector.tensor_copy(spd[par][o][:], zsb[:, 0:QL])

            CH0 = 16   # b0 output rows per chunk
            CH1 = 32   # b1 output rows per chunk
            pch = [wk.tile([128, CH0 * P0], F32R, tag=f"patch{j}",
                           name=f"patch{j}") for j in range(2)]
            nc.vector.tensor_copy(pch[0][:, :], zsb[:, 0:CH0 * P0])
            nc.vector.tensor_copy(pch[1][:, :], zsb[:, 0:CH0 * P0])

            taps_l = taps

            def emit_zx_otile(i, ot):
                ps = ppg.tile([128, SL], F32, tag="psg", name="psg")
                k = 0
                for ti in range(9):
                    ky, kx = taps_l[ti]
                    for ct in range(2):
                        nc.tensor.matmul(
                            ps[:, :], _gate_lhs(wxsb, ot, ti, ct),
                            spd[i % 2][ct][:, ky * PL + kx:
                                           ky * PL + kx + SL],
                            start=(k == 0), stop=(k == 17))
                        k += 1
                zs = wk.tile([128, SL], F32, tag="zxs", bufs=2, name="zxs")
                nc.scalar.activation(zs[:], ps[:, :], AF.Identity,
                                     bias=lbsb[:, ot:ot + 1])
                nc.sync.dma_start(zx[i, ot], zs[:])

            for i in range(IPC):
                # ---- b0: conv 1->64 via im2col (contract 9 pad 128);
                # interleave previous image's Zx0 otiles to keep PE dense --
                for c in range(128 // CH0):
                    if i > 0:
                        emit_zx_otile(i - 1, c)
                    patch = pch[c % 2]
                    # patch[3ky+kx, s] = xpad[i][(130ky+kx) + r0*130 + s]
                    for ky in range(3):
                        psrc = _ap(xpad, i * XPADN + c * CH0 * P0 + P0 * ky,
                                   [[1, 3], [1, CH0 * P0]])
                        nc.sync.dma_start(patch[3 * ky:3 * ky + 3, :], psrc)
                    y0 = wk.tile([64, CH0 * P0], F32, tag="y0")
                    nsub = (CH0 * P0 + 511) // 512
                    for s in range(nsub):
                        lo = s * 512
                        ln = min(512, CH0 * P0 - lo)
                        ps = pp.tile([128, 512], F32, tag="pss")
                        nc.tensor.matmul(ps[0:64, :ln], w0sb[:],
                                         patch[:, lo:lo + ln],
                                         start=True, stop=True)
                        if s % 2 == 0:
                            nc.scalar.activation(y0[:, lo:lo + ln],
                                                 ps[0:64, :ln],
                                                 AF.Relu, bias=b0sb[:, 0:1])
                        else:
                            nc.vector.scalar_tensor_tensor(
                                y0[:, lo:lo + ln], ps[0:64, :ln],
                                b0sb[:, 0:1], zsb[0:64, lo:lo + ln],
                                ALU.add, ALU.max)
                    # pool 2x2: span rows CH0 x 130, valid cols 1..128
                    y3 = y0[:].rearrange("p (r c) -> p r c", c=P0)
                    pa = wk.tile([64, CH0 * 64], F32, tag="pa")
                    pa3 = pa[:].rearrange("p (r c) -> p r c", c=64)
                    nc.vector.tensor_tensor(
                        pa3, y3[:, :, 1:129:2], y3[:, :, 2:130:2], ALU.max)
                    r0 = c * CH0 // 2
                    dst = _ap(x1p.tensor, x1p.offset + 68 + r0 * P1,
                              [[x1p.ap[0][0], 64], [P1, CH0 // 2],
                               [1, 64]])
                    nc.vector.tensor_tensor(
                        dst, pa3[:, 0:CH0:2, :], pa3[:, 1:CH0:2, :], ALU.max)

                # ---- b1: conv 64->128 + pool ----
                for c in range(64 // CH1):
                    base = c * CH1 * P1
                    y1 = wk.tile([128, CH1 * P1], F32, tag="y1", bufs=2)
                    nsub = (CH1 * P1 + 511) // 512
                    for s in range(nsub):
                        lo = s * 512
                        ln = min(512, CH1 * P1 - lo)
                        ps = pp.tile([128, 512], F32, tag="pss")
                        for ti, (ky, kx) in enumerate(taps):
                            off = ky * P1 + kx + base + lo
                            nc.tensor.matmul(
                                ps[:, :ln],
                                w1sb[:, ti * 128:(ti + 1) * 128],
                                x1p[:, off:off + ln],
                                start=(ti == 0), stop=(ti == 8))
                        nc.scalar.activation(y1[:, lo:lo + ln], ps[:, :ln],
                                             AF.Relu, bias=b1sb[:, 0:1])
                    y3 = y1[:].rearrange("p (r c) -> p r c", c=P1)
                    pa1 = wk.tile([128, CH1 * 32], F32, tag="pa1")
                    pa3 = pa1[:].rearrange("p (r c) -> p r c", c=32)
                    nc.vector.tensor_tensor(
                        pa3, y3[:, :, 1:65:2], y3[:, :, 2:66:2], ALU.max)
                    r0 = c * CH1 // 2
                    dst = _ap(x2p.tensor, x2p.offset + 36 + r0 * P2,
                              [[x2p.ap[0][0], 128], [P2, CH1 // 2],
                               [1, 32]])
                    nc.vector.tensor_tensor(
                        dst, pa3[:, 0:CH1:2, :], pa3[:, 1:CH1:2, :], ALU.max)

                # ---- b2: conv 128->256 + pool ----
                for o in range(2):
                    y2 = wk.tile([128, S2], F32, tag="y2")
                    nsub = (S2 + 511) // 512
                    for s in range(nsub):
                        lo = s * 512
                        ln = min(512, S2 - lo)
                        ps = pp.tile([128, 512], F32, tag="pss")
                        for ti, (ky, kx) in enumerate(taps):
                            off = ky * P2 + kx + lo
                            nc.tensor.matmul(
                                ps[:, :ln],
                                w2sb[:, ti * 256 + o * 128:
                                     ti * 256 + o * 128 + 128],
                                x2p[:, off:off + ln],
                                start=(ti == 0), stop=(ti == 8))
                        nc.scalar.activation(y2[:, lo:lo + ln], ps[:, :ln],
                                             AF.Relu, bias=b2sb[:, o:o + 1])
                    y3 = y2[:].rearrange("p (r c) -> p r c", c=P2)
                    pa2 = wk.tile([128, 32 * 16], F32, tag="pa2")
                    pa3 = pa2[:].rearrange("p (r c) -> p r c", c=16)
                    nc.vector.tensor_tensor(
                        pa3, y3[:, :, 1:33:2], y3[:, :, 2:34:2], ALU.max)
                    sp = spd[i % 2][o]
                    dst = _ap(sp.tensor, sp.offset + 20,
                              [[sp.ap[0][0], 128], [PL, 16], [1, 16]])
                    nc.vector.tensor_tensor(
                        dst, pa3[:, 0:32:2, :], pa3[:, 1:32:2, :], ALU.max)

            # ---- Zx0 for the last image ----
            for ot in range(8):
                emit_zx_otile(IPC - 1, ot)
    nc.compile()
    return nc


# --------------------------------------------------------------------------
# Launch R: ConvLSTM recurrence (one layer, one batch element per core)
# --------------------------------------------------------------------------

def build_R():
    nc = bacc.Bacc("TRN2", target_bir_lowering=False, debug=False,
                   num_devices=N_CORES)
    zx = nc.dram_tensor("zx", [T, 128, 8 * SL], F32, kind="ExternalInput")
    wh = nc.dram_tensor("wh", [128, 8 * 2304], F32R, kind="ExternalInput")
    hpad_seq = nc.dram_tensor("hpad_seq", [T, 2, 128, QL], F32R,
                              kind="ExternalOutput")
    hval_seq = nc.dram_tensor("hval_seq", [T, 2, 128, 256], F32,
                              kind="ExternalOutput")

    # gate channel blocks: i: ot 0-1, f: 2-3, o: 4-5, g: 6-7
    with TileContext(nc) as tc:
        with (
            tc.tile_pool(name="wpool", bufs=1) as wp,
            tc.tile_pool(name="state", bufs=1) as stp,
            tc.tile_pool(name="work", bufs=2) as wk,
            tc.tile_pool(name="psum", bufs=6, space="PSUM") as pp,
        ):
            whsb = _load_gate_weights(nc, wp, wh,
                                      order=(6, 7, 2, 3, 0, 1, 4, 5))
            hp = [stp.tile([128, QL], F32R, tag=f"hp{c}", name=f"hp{c}")
                  for c in range(2)]
            cs = [stp.tile([128, SL], F32, tag=f"cs{c}", name=f"cs{c}")
                  for c in range(2)]
            zsb = stp.tile([128, QL], F32)
            nc.vector.memset(zsb[:], 0.0)
            for c in range(2):
                nc.vector.tensor_copy(hp[c][:], zsb[:, 0:QL])
                nc.vector.memset(cs[c][:], 0.0)

            taps = _taps()
            for t in range(T):
                zxsb = wk.tile([128, 8 * SL], F32, tag="zxsb")
                nc.sync.dma_start(zxsb[:], zx[t])

                # otile order: g gates (6,7) first, then f (2,3), i (0,1),
                # o last (4,5) -- lets the c-update run under the remaining
                # convs so the serial tail is only sigma(o), h, and the copy.
                ga = {}
                hss = {}
                for ot in (6, 7, 2, 3, 0, 1, 4, 5):
                    ps = pp.tile([128, SL], F32, tag="psr")
                    k = 0
                    for ti, (ky, kx) in enumerate(taps):
                        for ct in range(2):
                            toff = ky * PL + kx
                            nc.tensor.matmul(ps[:, :],
                                             _gate_lhs(whsb, ot, ti, ct),
                                             hp[ct][:, toff:toff + SL],
                                             start=(k == 0), stop=(k == 17))
                            k += 1
                    zt = wk.tile([128, SL], F32, tag=f"zt{ot}", name=f"zt{ot}")
                    nc.vector.tensor_tensor(zt[:], ps[:, :],
                                            zxsb[:, ot * SL:(ot + 1) * SL],
                                            ALU.add)
                    g = wk.tile([128, SL], F32, tag=f"ga{ot}", name=f"ga{ot}")
                    nc.scalar.activation(
                        g[:], zt[:], AF.Tanh if ot >= 6 else AF.Sigmoid)
                    ga[ot] = g

                    if ot in (0, 1):
                        # i gate done; g, f already done: c = f*c + i*g
                        ht = ot
                        t1 = wk.tile([128, SL], F32, tag=f"t1{ht}",
                                     name=f"t1{ht}")
                        nc.vector.tensor_tensor(t1[:], ga[2 + ht][:],
                                                cs[ht][:], ALU.mult)
                        t2 = wk.tile([128, SL], F32, tag=f"t2{ht}",
                                     name=f"t2{ht}")
                        nc.vector.tensor_tensor(t2[:], ga[ht][:],
                                                ga[6 + ht][:], ALU.mult)
                        nc.vector.tensor_tensor(cs[ht][:], t1[:], t2[:],
                                                ALU.add)
                        th = wk.tile([128, SL], F32, tag=f"th{ht}",
                                     name=f"th{ht}")
                        nc.scalar.activation(th[:], cs[ht][:], AF.Tanh)
                        hss[ht] = th
                    if ot in (4, 5):
                        ht = ot - 4
                        hs = wk.tile([128, SL], F32, tag=f"hs{ht}",
                                     name=f"hs{ht}")
                        nc.vector.tensor_tensor(hs[:], ga[ot][:],
                                                hss[ht][:], ALU.mult)
                        hss[ht] = hs
                # hp may only be overwritten after ALL 8 otile convs of this
                # step have read h_{t-1}; emit the copies after the loop.
                for ht in range(2):
                    hs = hss[ht]
                    hpt = hp[ht]
                    dst = _ap(hpt.tensor, hpt.offset + 20,
                              [[hpt.ap[0][0], 128], [PL, 16], [1, 16]])
                    src = _ap(hs.tensor, hs.offset + 1,
                              [[hs.ap[0][0], 128], [PL, 16], [1, 16]])
                    nc.vector.tensor_copy(dst, src)
                    nc.sync.dma_start(hpad_seq[t, ht], hpt[:])
                    nc.sync.dma_start(hval_seq[t, ht], src)
    nc.compile()
    return nc


# --------------------------------------------------------------------------
# Launch C: layer-1 input-gate conv over layer-0 hidden states
# --------------------------------------------------------------------------

def build_C():
    nc = bacc.Bacc("TRN2", target_bir_lowering=False, debug=False,
                   num_devices=N_CORES)
    hpad = nc.dram_tensor("hpad", [IPC, 128, 2 * QL], F32R,
                          kind="ExternalInput")
    wx = nc.dram_tensor("wx", [128, 8 * 2304], F32R, kind="ExternalInput")
    lb = nc.dram_tensor("lb", [128, 8], F32, kind="ExternalInput")
    zx = nc.dram_tensor("zx", [IPC, 8, 128, SL], F32, kind="ExternalOutput")

    with TileContext(nc) as tc:
        with (
            tc.tile_pool(name="wpool", bufs=1) as wp,
            tc.tile_pool(name="work", bufs=2) as wk,
            tc.tile_pool(name="psum", bufs=4, space="PSUM") as pp,
        ):
            wxsb = _load_gate_weights(nc, wp, wx)
            lbsb = wp.tile([128, 8], F32)
            nc.sync.dma_start(lbsb[:], lb[:, :])
            for i in range(IPC):
                hsb = wk.tile([128, 2 * QL], F32R, tag="hsb")
                nc.sync.dma_start(hsb[:], hpad[i])

                def out_cb(ot, ps, i=i):
                    zs = wk.tile([128, SL], F32, tag="zxs")
                    nc.scalar.activation(zs[:], ps[:, :], AF.Identity,
                                         bias=lbsb[:, ot:ot + 1])
                    nc.sync.dma_start(zx[i, ot], zs[:])

                _emit_gate_conv(
                    nc, pp, wxsb,
                    lambda ct, toff, hsb=hsb: hsb[:, ct * QL + toff:
                                                  ct * QL + toff + SL],
                    out_cb)
    nc.compile()
    return nc


# --------------------------------------------------------------------------
# host orchestration
# --------------------------------------------------------------------------

def _get_programs():
    if not _PROGRAMS:
        _PROGRAMS["A"] = build_A()
        _PROGRAMS["R"] = build_R()
        _PROGRAMS["C"] = build_C()
    return _PROGRAMS


def _enable_profiling():
    import sys
    import types
    import concourse.bass_utils as bu
    bu.upload_artifacts = lambda tmpdir: tmpdir
    if "antenv.axon_hooks" in sys.modules:
        return
    try:
        from trn_agent_boot.trn_boot import _ntff_profile_via_ctypes
        hook = _ntff_profile_via_ctypes("/opt/axon/libaxon_pjrt.so")
        m = types.ModuleType("antenv.axon_hooks")
        m.get_axon_ntff_profile_hook = lambda: hook
        sys.modules["antenv.axon_hooks"] = m
    except Exception:
        pass


def _run(nc, in_maps):
    global LAST_EXEC_NS
    if PROFILE:
        _enable_profiling()
        res = run_bass_kernel_spmd(nc, in_maps, list(range(N_CORES)),
                                   trace=True)
        if res.exec_time_ns is not None:
            LAST_EXEC_NS += res.exec_time_ns
    else:
        res = run_bass_kernel_spmd(nc, in_maps, list(range(N_CORES)))
    return res.results


def _gate_weight_pack(lw, lo):
    """lw [1024, cin+256, 3, 3] -> (wx, wh) each [128, 8*2304]:
    wsb[p, o*2304 + (ti*2+ct)*128 + j] = W[o*128+j, base+ct*128+p, ky, kx]."""
    cin = lw.shape[1] - HID
    out = []
    for base in (0, cin):
        w = np.empty((9, 2, 128, 1024), np.float32)
        for ti, (ky, kx) in enumerate(_taps()):
            for ct in range(2):
                sl = lw[:, base + ct * 128: base + (ct + 1) * 128, ky, kx]
                w[ti, ct] = np.ascontiguousarray(sl.T)
        packed = np.ascontiguousarray(
            w.reshape(9, 2, 128, 8, 128).transpose(2, 3, 0, 1, 4)
        ).reshape(128, 8 * 2304)
        out.append(packed)
    return out[0], out[1]


def kernel(x, w0, b0, g0, be0, m0, v0,
           w1, b1, g1, be1, m1, v1,
           w2, b2, g2, be2, m2, v2,
           lw0, lb0, lw1, lb1):
    global LAST_EXEC_NS
    LAST_EXEC_NS = 0.0
    progs = _get_programs()

    x = np.asarray(x, np.float32)
    nB, nT = x.shape[0], x.shape[1]
    assert (nB, nT) == (B, T)

    # ---- host prep: padded images ----
    imgs = x.reshape(IMGS, H, W)
    xpad = np.zeros((IMGS, XPADN), np.float32)
    v = xpad[:, 1:1 + P0 * P0].reshape(IMGS, P0, P0)
    v[:, 1:129, 1:129] = imgs

    # ---- stem weights with BN folding ----
    def fold(w, b, g, be, m, vv):
        scale = (np.asarray(g) / np.sqrt(np.asarray(vv) + EPS)).astype(np.float32)
        shift = (np.asarray(be) - np.asarray(m) * scale).astype(np.float32)
        bias = np.asarray(b) * scale + shift
        wt = np.asarray(w) * scale[:, None, None, None]
        return wt.astype(np.float32), bias.astype(np.float32)

    w0f, bias0 = fold(w0, b0, g0, be0, m0, v0)
    w1f, bias1 = fold(w1, b1, g1, be1, m1, v1)
    w2f, bias2 = fold(w2, b2, g2, be2, m2, v2)

    w0t = np.zeros((128, 64), np.float32)
    w1t = np.zeros((9, 128, 128), np.float32)
    w2t = np.empty((9, 128, 256), np.float32)
    for ti, (ky, kx) in enumerate(_taps()):
        w0t[ti] = w0f[:, 0, ky, kx]
        w1t[ti, 0:64] = w1f[:, :, ky, kx].T
        w2t[ti] = w2f[:, :, ky, kx].T
    b0a = bias0.reshape(64, 1)
    b1a = bias1.reshape(128, 1)
    b2a = np.ascontiguousarray(bias2.reshape(2, 128).T)

    wx0, wh0 = _gate_weight_pack(np.asarray(lw0, np.float32), lb0)
    wx1, wh1 = _gate_weight_pack(np.asarray(lw1, np.float32), lb1)
    lb0a = np.ascontiguousarray(np.asarray(lb0, np.float32).reshape(8, 128).T)
    lb1a = np.ascontiguousarray(np.asarray(lb1, np.float32).reshape(8, 128).T)

    # ---- launch A ----
    in_a = [{"xpad": xpad[c * IPC:(c + 1) * IPC], "w0t": w0t, "b0t": b0a,
             "w1t": w1t, "b1t": b1a, "w2t": w2t, "b2t": b2a,
             "wx": wx0, "lb": lb0a} for c in range(N_CORES)]
    res = _run(progs["A"], in_a)
    zx0 = np.concatenate([res[c]["zx"] for c in range(N_CORES)], axis=0)
    # zx0: [32, 8, 128, SL], img index = b*T + t

    # ---- launch R (layer 0) ----
    zxr0 = [np.ascontiguousarray(
        zx0[b_ * T:(b_ + 1) * T].transpose(0, 2, 1, 3)).reshape(T, 128, 8 * SL)
        for b_ in range(B)]
    in_r = [{"zx": zxr0[c % B], "wh": wh0} for c in range(N_CORES)]
    res = _run(progs["R"], in_r)
    hpad0 = [res[b]["hpad_seq"] for b in range(B)]  # [T, 2, 128, QL] each

    # ---- launch C (Zx1) ----
    hp_items = np.ascontiguousarray(
        np.concatenate(hpad0, axis=0).transpose(0, 2, 1, 3)
    ).reshape(IMGS, 128, 2 * QL)
    in_c = [{"hpad": hp_items[c * IPC:(c + 1) * IPC], "wx": wx1, "lb": lb1a}
            for c in range(N_CORES)]
    res = _run(progs["C"], in_c)
    zx1 = np.concatenate([res[c]["zx"] for c in range(N_CORES)], axis=0)

    # ---- launch R (layer 1) ----
    zxr1 = [np.ascontiguousarray(
        zx1[b_ * T:(b_ + 1) * T].transpose(0, 2, 1, 3)).reshape(T, 128, 8 * SL)
        for b_ in range(B)]
    in_r = [{"zx": zxr1[c % B], "wh": wh1} for c in range(N_CORES)]
    res = _run(progs["R"], in_r)

    z_seq = np.empty((B, T, HID, 16, 16), np.float32)
    for b_ in range(B):
        hv = res[b_]["hval_seq"]  # [T, 2, 128, 256]
        z_seq[b_] = hv.reshape(T, HID, 16, 16)
    z_last = np.ascontiguousarray(z_seq[:, -1])
    return z_seq, z_last


# revision 17
# speedup vs baseline: 1.5118x; 1.0049x over previous
"""Trainium2 Bass kernel for nn_Encoder (CNN stem + 2-layer ConvLSTM).

Self-contained: builds three Bass programs (compiled once per process) and
orchestrates four SPMD launches on 8 NeuronCores:

  A: CNN stem (3x conv+BN+ReLU+pool) + layer-0 input-gate conv Zx0,
     data-parallel over the 32 (b,t) images (4 per core).
  R: ConvLSTM recurrence (16 sequential steps; hidden-half gate conv +
     gate nonlinearities + state update), data-parallel over batch (B=2).
     Compiled once, launched twice (layer 0 and layer 1).
  C: layer-1 input-gate conv Zx1 over layer-0 hidden states,
     data-parallel over the 32 (b,t) items.

All matmuls run as float32r (TF32-like, ~1 PE cycle/row at free>=256).
Conv3x3 is 9 shifted matmuls accumulated in PSUM over zero-padded
flat buffers with a 1-element lead/tail pad; the pad columns of each
conv output span carry junk that is never read by valid outputs.
"""

import numpy as np

import concourse.bass as bass
import concourse.mybir as mybir
from concourse import bacc
from concourse.bass_utils import run_bass_kernel_spmd
from concourse.tile import TileContext

F32 = mybir.dt.float32
F32R = mybir.dt.float32r
AF = mybir.ActivationFunctionType
ALU = mybir.AluOpType

N_CORES = 8
B, T, H, W = 2, 16, 128, 128
HID = 256
EPS = 1e-5
IMGS = B * T           # 32
IPC = IMGS // N_CORES  # 4 images per core

# stem geometry: (rows, padded width, span length, lead-padded buffer len)
# level 0 input: 128x128 -> padded 130x130
P0 = 130
XPADN = P0 * P0 + 2          # host-side lead/tail padded flat image
# level 1 input: 64x64 -> padded 66
P1, S1 = 66, 64 * 66         # span covers 64 rows x 66 cols
Q1 = 66 * 66 + 2
# level 2 input: 32x32 -> padded 34
P2, S2 = 34, 32 * 34
Q2 = 34 * 34 + 2
# lstm spatial: 16x16 -> padded 18
PL, SL = 18, 16 * 18         # SL = 288
QL = 18 * 18 + 2             # 326

PROFILE = False
LAST_EXEC_NS = 0.0

_PROGRAMS = {}


def _taps():
    return [(ky, kx) for ky in range(3) for kx in range(3)]


def _ap(handle, offset, dims):
    return bass.AP(handle, offset, [list(d) for d in dims])


# --------------------------------------------------------------------------
# shared emitters
# --------------------------------------------------------------------------

def _gate_lhs(wsb, ot, ti, ct, n_ct=2):
    base = ot * 9 * n_ct * 128 + (ti * n_ct + ct) * 128
    return wsb[:, base:base + 128]


def _emit_gate_conv(nc, psum_pool, wsb, src_slices, out_cb, n_ct=2,
                    order=tuple(range(8))):
    """z[ot] = sum_{tap,ct} W[tap,ct,:,ot*128:+128].T @ src[ct][tapoff:+288]."""
    taps = _taps()
    for ot in order:
        ps = psum_pool.tile([128, SL], F32, tag="psg")
        n_k = len(taps) * n_ct
        k = 0
        for ti, (ky, kx) in enumerate(taps):
            for ct in range(n_ct):
                nc.tensor.matmul(ps[:, :], _gate_lhs(wsb, ot, ti, ct, n_ct),
                                 src_slices(ct, ky * PL + kx),
                                 start=(k == 0), stop=(k == n_k - 1))
                k += 1
        out_cb(ot, ps)


def _load_gate_weights(nc, sb, w_dram, n_ct=2, order=tuple(range(8))):
    """DMA host-prepacked [128, 8*9*n_ct*128] f32r weights (otile-major,
    contiguous slabs) into SBUF. First-used slab goes on the scalar HWDGE
    queue (fast, uncontended); the rest stream on SWDGE in use-order."""
    slab = 9 * n_ct * 128
    wsb = sb.tile([128, 8 * slab], F32R, tag="wgate")
    for j, o in enumerate(order):
        eng = nc.scalar if j == 0 else nc.gpsimd
        eng.dma_start(wsb[:, o * slab:(o + 1) * slab],
                      w_dram[:, o * slab:(o + 1) * slab])
    return wsb


# --------------------------------------------------------------------------
# Launch A: CNN stem + Zx0
# --------------------------------------------------------------------------

def build_A():
    nc = bacc.Bacc("TRN2", target_bir_lowering=False, debug=False,
                   num_devices=N_CORES)
    xpad = nc.dram_tensor("xpad", [IPC, XPADN], F32R, kind="ExternalInput")
    w0t = nc.dram_tensor("w0t", [128, 64], F32R, kind="ExternalInput")
    b0t = nc.dram_tensor("b0t", [64, 1], F32, kind="ExternalInput")
    w1t = nc.dram_tensor("w1t", [9, 128, 128], F32R, kind="ExternalInput")
    b1t = nc.dram_tensor("b1t", [128, 1], F32, kind="ExternalInput")
    w2t = nc.dram_tensor("w2t", [9, 128, 256], F32R, kind="ExternalInput")
    b2t = nc.dram_tensor("b2t", [128, 2], F32, kind="ExternalInput")
    wx = nc.dram_tensor("wx", [128, 8 * 2304], F32R, kind="ExternalInput")
    lb = nc.dram_tensor("lb", [128, 8], F32, kind="ExternalInput")
    zx = nc.dram_tensor("zx", [IPC, 8, 128, SL], F32, kind="ExternalOutput")

    taps = _taps()
    with TileContext(nc) as tc:
        with (
            tc.tile_pool(name="wpool", bufs=1) as wp,
            tc.tile_pool(name="pads", bufs=1) as padp,
            tc.tile_pool(name="work", bufs=1) as wk,
            tc.tile_pool(name="psum", bufs=3, space="PSUM") as pp,
            tc.tile_pool(name="psumg", bufs=5, space="PSUM") as ppg,
        ):
            w0sb = wp.tile([128, 64], F32R)
            nc.sync.dma_start(w0sb[:], w0t[:, :])
            w1sb = wp.tile([128, 9 * 128], F32R)
            nc.gpsimd.dma_start(
                w1sb[:].rearrange("p (t o) -> p t o", t=9),
                _ap(w1t, 0, [[128, 128], [128 * 128, 9], [1, 128]]))
            w2sb = wp.tile([128, 9 * 256], F32R)
            nc.gpsimd.dma_start(
                w2sb[:].rearrange("p (t o) -> p t o", t=9),
                _ap(w2t, 0, [[256, 128], [128 * 256, 9], [1, 256]]))
            b0sb = wp.tile([64, 1], F32)
            nc.sync.dma_start(b0sb[:], b0t[:, :])
            b1sb = wp.tile([128, 1], F32)
            nc.sync.dma_start(b1sb[:], b1t[:, :])
            b2sb = wp.tile([128, 2], F32)
            nc.sync.dma_start(b2sb[:], b2t[:, :])
            lbsb = wp.tile([128, 8], F32)
            nc.sync.dma_start(lbsb[:], lb[:, :])
            wxsb = _load_gate_weights(nc, wp, wx)

            x1p = padp.tile([128, Q1], F32R)
            x2p = padp.tile([128, Q2], F32R)
            spd = [[padp.tile([128, QL], F32R, tag=f"spd{o}{par}",
                              name=f"spd{o}{par}") for o in range(2)]
                   for par in range(2)]
            zsb = padp.tile([128, Q1], F32)
            nc.vector.memset(zsb[:], 0.0)
            nc.vector.tensor_copy(x1p[:], zsb[:, 0:Q1])
            nc.scalar.copy(x2p[:], zsb[:, 0:Q2])
            for par in range(2):
                for o in range(2):
                    nc.scalar.copy(spd[par][o][:], zsb[:, 0:QL])

            CH0 = 16   # b0 output rows per chunk
            CH1 = 32   # b1 output rows per chunk
            pch = [wk.tile([128, CH0 * P0], F32R, tag=f"patch{j}",
                           name=f"patch{j}") for j in range(2)]
            nc.vector.tensor_copy(pch[0][:, :], zsb[:, 0:CH0 * P0])
            nc.vector.tensor_copy(pch[1][:, :], zsb[:, 0:CH0 * P0])

            taps_l = taps

            def emit_zx_otile(i, ot):
                ps = ppg.tile([128, SL], F32, tag="psg", name="psg")
                k = 0
                for ti in range(9):
                    ky, kx = taps_l[ti]
                    for ct in range(2):
                        nc.tensor.matmul(
                            ps[:, :], _gate_lhs(wxsb, ot, ti, ct),
                            spd[i % 2][ct][:, ky * PL + kx:
                                           ky * PL + kx + SL],
                            start=(k == 0), stop=(k == 17))
                        k += 1
                zs = wk.tile([128, SL], F32, tag="zxs", bufs=2, name="zxs")
                nc.scalar.activation(zs[:], ps[:, :], AF.Identity,
                                     bias=lbsb[:, ot:ot + 1])
                nc.sync.dma_start(zx[i, ot], zs[:])

            for i in range(IPC):
                # ---- b0: conv 1->64 via im2col (contract 9 pad 128);
                # interleave previous image's Zx0 otiles to keep PE dense --
                for c in range(128 // CH0):
                    if i > 0:
                        emit_zx_otile(i - 1, c)
                    else:
                        # no previous image to interleave: keep the PE busy
                        # through b0's DMA/evict stalls so HAM stays warm
                        for d in range(8):
                            pd = ppg.tile([128, SL], F32, tag="psg",
                                          name="psg")
                            nc.tensor.matmul(pd[0:64, :], w0sb[:, 0:64],
                                             pch[0][:, 0:SL],
                                             start=True, stop=True)
                    patch = pch[c % 2]
                    # patch[3ky+kx, s] = xpad[i][(130ky+kx) + r0*130 + s]
                    for ky in range(3):
                        psrc = _ap(xpad, i * XPADN + c * CH0 * P0 + P0 * ky,
                                   [[1, 3], [1, CH0 * P0]])
                        nc.sync.dma_start(patch[3 * ky:3 * ky + 3, :], psrc)
                    y0 = wk.tile([64, CH0 * P0], F32, tag="y0")
                    nsub = (CH0 * P0 + 511) // 512
                    for s in range(nsub):
                        lo = s * 512
                        ln = min(512, CH0 * P0 - lo)
                        ps = pp.tile([128, 512], F32, tag="pss")
                        nc.tensor.matmul(ps[0:64, :ln], w0sb[:],
                                         patch[:, lo:lo + ln],
                                         start=True, stop=True)
                        if s % 2 == 0:
                            nc.scalar.activation(y0[:, lo:lo + ln],
                                                 ps[0:64, :ln],
                                                 AF.Relu, bias=b0sb[:, 0:1])
                        else:
                            nc.vector.scalar_tensor_tensor(
                                y0[:, lo:lo + ln], ps[0:64, :ln],
                                b0sb[:, 0:1], zsb[0:64, lo:lo + ln],
                                ALU.add, ALU.max)
                    # pool 2x2: span rows CH0 x 130, valid cols 1..128
                    y3 = y0[:].rearrange("p (r c) -> p r c", c=P0)
                    pa = wk.tile([64, CH0 * 64], F32, tag="pa")
                    pa3 = pa[:].rearrange("p (r c) -> p r c", c=64)
                    nc.vector.tensor_tensor(
                        pa3, y3[:, :, 1:129:2], y3[:, :, 2:130:2], ALU.max)
                    r0 = c * CH0 // 2
                    dst = _ap(x1p.tensor, x1p.offset + 68 + r0 * P1,
                              [[x1p.ap[0][0], 64], [P1, CH0 // 2],
                               [1, 64]])
                    nc.vector.tensor_tensor(
                        dst, pa3[:, 0:CH0:2, :], pa3[:, 1:CH0:2, :], ALU.max)

                # ---- b1: conv 64->128 + pool ----
                for c in range(64 // CH1):
                    base = c * CH1 * P1
                    y1 = wk.tile([128, CH1 * P1], F32, tag="y1", bufs=2)
                    nsub = (CH1 * P1 + 511) // 512
                    for s in range(nsub):
                        lo = s * 512
                        ln = min(512, CH1 * P1 - lo)
                        ps = pp.tile([128, 512], F32, tag="pss")
                        for ti, (ky, kx) in enumerate(taps):
                            off = ky * P1 + kx + base + lo
                            nc.tensor.matmul(
                                ps[:, :ln],
                                w1sb[:, ti * 128:(ti + 1) * 128],
                                x1p[:, off:off + ln],
                                start=(ti == 0), stop=(ti == 8))
                        nc.scalar.activation(y1[:, lo:lo + ln], ps[:, :ln],
                                             AF.Relu, bias=b1sb[:, 0:1])
                    y3 = y1[:].rearrange("p (r c) -> p r c", c=P1)
                    pa1 = wk.tile([128, CH1 * 32], F32, tag="pa1")
                    pa3 = pa1[:].rearrange("p (r c) -> p r c", c=32)
                    nc.vector.tensor_tensor(
                        pa3, y3[:, :, 1:65:2], y3[:, :, 2:66:2], ALU.max)
                    r0 = c * CH1 // 2
                    dst = _ap(x2p.tensor, x2p.offset + 36 + r0 * P2,
                              [[x2p.ap[0][0], 128], [P2, CH1 // 2],
                               [1, 32]])
                    nc.vector.tensor_tensor(
                        dst, pa3[:, 0:CH1:2, :], pa3[:, 1:CH1:2, :], ALU.max)

                # ---- b2: conv 128->256 + pool ----
                for o in range(2):
                    y2 = wk.tile([128, S2], F32, tag="y2")
                    nsub = (S2 + 511) // 512
                    for s in range(nsub):
                        lo = s * 512
                        ln = min(512, S2 - lo)
                        ps = pp.tile([128, 512], F32, tag="pss")
                        for ti, (ky, kx) in enumerate(taps):
                            off = ky * P2 + kx + lo
                            nc.tensor.matmul(
                                ps[:, :ln],
                                w2sb[:, ti * 256 + o * 128:
                                     ti * 256 + o * 128 + 128],
                                x2p[:, off:off + ln],
                                start=(ti == 0), stop=(ti == 8))
                        nc.scalar.activation(y2[:, lo:lo + ln], ps[:, :ln],
                                             AF.Relu, bias=b2sb[:, o:o + 1])
                    y3 = y2[:].rearrange("p (r c) -> p r c", c=P2)
                    pa2 = wk.tile([128, 32 * 16], F32, tag="pa2")
                    pa3 = pa2[:].rearrange("p (r c) -> p r c", c=16)
                    nc.vector.tensor_tensor(
                        pa3, y3[:, :, 1:33:2], y3[:, :, 2:34:2], ALU.max)
                    sp = spd[i % 2][o]
                    dst = _ap(sp.tensor, sp.offset + 20,
                              [[sp.ap[0][0], 128], [PL, 16], [1, 16]])
                    nc.vector.tensor_tensor(
                        dst, pa3[:, 0:32:2, :], pa3[:, 1:32:2, :], ALU.max)

            # ---- Zx0 for the last image ----
            for ot in range(8):
                emit_zx_otile(IPC - 1, ot)
    nc.compile()
    return nc


# --------------------------------------------------------------------------
# Launch R: ConvLSTM recurrence (one layer, one batch element per core)
# --------------------------------------------------------------------------

def build_R():
    nc = bacc.Bacc("TRN2", target_bir_lowering=False, debug=False,
                   num_devices=N_CORES)
    zx = nc.dram_tensor("zx", [T, 128, 8 * SL], F32, kind="ExternalInput")
    wh = nc.dram_tensor("wh", [128, 8 * 2304], F32R, kind="ExternalInput")
    hpad_seq = nc.dram_tensor("hpad_seq", [T, 2, 128, QL], F32R,
                              kind="ExternalOutput")
    hval_seq = nc.dram_tensor("hval_seq", [T, 2, 128, 256], F32,
                              kind="ExternalOutput")

    # gate channel blocks: i: ot 0-1, f: 2-3, o: 4-5, g: 6-7
    with TileContext(nc) as tc:
        with (
            tc.tile_pool(name="wpool", bufs=1) as wp,
            tc.tile_pool(name="state", bufs=1) as stp,
            tc.tile_pool(name="work", bufs=2) as wk,
            tc.tile_pool(name="psum", bufs=6, space="PSUM") as pp,
        ):
            whsb = _load_gate_weights(nc, wp, wh,
                                      order=(6, 7, 2, 3, 0, 1, 4, 5))
            hp = [stp.tile([128, QL], F32R, tag=f"hp{c}", name=f"hp{c}")
                  for c in range(2)]
            cs = [stp.tile([128, SL], F32, tag=f"cs{c}", name=f"cs{c}")
                  for c in range(2)]
            zsb = stp.tile([128, QL], F32)
            nc.vector.memset(zsb[:], 0.0)
            for c in range(2):
                nc.vector.tensor_copy(hp[c][:], zsb[:, 0:QL])
                nc.vector.memset(cs[c][:], 0.0)

            taps = _taps()
            for t in range(T):
                zxsb = wk.tile([128, 8 * SL], F32, tag="zxsb")
                nc.sync.dma_start(zxsb[:], zx[t])

                # otile order: g gates (6,7) first, then f (2,3), i (0,1),
                # o last (4,5) -- lets the c-update run under the remaining
                # convs so the serial tail is only sigma(o), h, and the copy.
                ga = {}
                hss = {}
                for ot in (6, 7, 2, 3, 0, 1, 4, 5):
                    ps = pp.tile([128, SL], F32, tag="psr")
                    k = 0
                    for ti, (ky, kx) in enumerate(taps):
                        for ct in range(2):
                            toff = ky * PL + kx
                            nc.tensor.matmul(ps[:, :],
                                             _gate_lhs(whsb, ot, ti, ct),
                                             hp[ct][:, toff:toff + SL],
                                             start=(k == 0), stop=(k == 17))
                            k += 1
                    zt = wk.tile([128, SL], F32, tag=f"zt{ot}", name=f"zt{ot}")
                    nc.vector.tensor_tensor(zt[:], ps[:, :],
                                            zxsb[:, ot * SL:(ot + 1) * SL],
                                            ALU.add)
                    g = wk.tile([128, SL], F32, tag=f"ga{ot}", name=f"ga{ot}")
                    nc.scalar.activation(
                        g[:], zt[:], AF.Tanh if ot >= 6 else AF.Sigmoid)
                    ga[ot] = g

                    if ot in (0, 1):
                        # i gate done; g, f already done: c = f*c + i*g
                        ht = ot
                        t1 = wk.tile([128, SL], F32, tag=f"t1{ht}",
                                     name=f"t1{ht}")
                        nc.vector.tensor_tensor(t1[:], ga[2 + ht][:],
                                                cs[ht][:], ALU.mult)
                        t2 = wk.tile([128, SL], F32, tag=f"t2{ht}",
                                     name=f"t2{ht}")
                        nc.vector.tensor_tensor(t2[:], ga[ht][:],
                                                ga[6 + ht][:], ALU.mult)
                        nc.vector.tensor_tensor(cs[ht][:], t1[:], t2[:],
                                                ALU.add)
                        th = wk.tile([128, SL], F32, tag=f"th{ht}",
                                     name=f"th{ht}")
                        nc.scalar.activation(th[:], cs[ht][:], AF.Tanh)
                        hss[ht] = th
                    if ot in (4, 5):
                        ht = ot - 4
                        hs = wk.tile([128, SL], F32, tag=f"hs{ht}",
                                     name=f"hs{ht}")
                        nc.vector.tensor_tensor(hs[:], ga[ot][:],
                                                hss[ht][:], ALU.mult)
                        hss[ht] = hs
                # hp may only be overwritten after ALL 8 otile convs of this
                # step have read h_{t-1}; emit the copies after the loop.
                for ht in range(2):
                    hs = hss[ht]
                    hpt = hp[ht]
                    dst = _ap(hpt.tensor, hpt.offset + 20,
                              [[hpt.ap[0][0], 128], [PL, 16], [1, 16]])
                    src = _ap(hs.tensor, hs.offset + 1,
                              [[hs.ap[0][0], 128], [PL, 16], [1, 16]])
                    nc.vector.tensor_copy(dst, src)
                    nc.sync.dma_start(hpad_seq[t, ht], hpt[:])
                    nc.sync.dma_start(hval_seq[t, ht], src)
    nc.compile()
    return nc


# --------------------------------------------------------------------------
# Launch C: layer-1 input-gate conv over layer-0 hidden states
# --------------------------------------------------------------------------

def build_C():
    nc = bacc.Bacc("TRN2", target_bir_lowering=False, debug=False,
                   num_devices=N_CORES)
    hpad = nc.dram_tensor("hpad", [IPC, 128, 2 * QL], F32R,
                          kind="ExternalInput")
    wx = nc.dram_tensor("wx", [128, 8 * 2304], F32R, kind="ExternalInput")
    lb = nc.dram_tensor("lb", [128, 8], F32, kind="ExternalInput")
    zx = nc.dram_tensor("zx", [IPC, 8, 128, SL], F32, kind="ExternalOutput")

    with TileContext(nc) as tc:
        with (
            tc.tile_pool(name="wpool", bufs=1) as wp,
            tc.tile_pool(name="work", bufs=2) as wk,
            tc.tile_pool(name="psum", bufs=4, space="PSUM") as pp,
        ):
            wxsb = _load_gate_weights(nc, wp, wx)
            lbsb = wp.tile([128, 8], F32)
            nc.sync.dma_start(lbsb[:], lb[:, :])
            for i in range(IPC):
                hsb = wk.tile([128, 2 * QL], F32R, tag="hsb")
                nc.sync.dma_start(hsb[:], hpad[i])

                def out_cb(ot, ps, i=i):
                    zs = wk.tile([128, SL], F32, tag="zxs")
                    nc.scalar.activation(zs[:], ps[:, :], AF.Identity,
                                         bias=lbsb[:, ot:ot + 1])
                    nc.sync.dma_start(zx[i, ot], zs[:])

                _emit_gate_conv(
                    nc, pp, wxsb,
                    lambda ct, toff, hsb=hsb: hsb[:, ct * QL + toff:
                                                  ct * QL + toff + SL],
                    out_cb)
    nc.compile()
    return nc


# --------------------------------------------------------------------------
# host orchestration
# --------------------------------------------------------------------------

def _get_programs():
    if not _PROGRAMS:
        _PROGRAMS["A"] = build_A()
        _PROGRAMS["R"] = build_R()
        _PROGRAMS["C"] = build_C()
    return _PROGRAMS


def _enable_profiling():
    import sys
    import types
    import concourse.bass_utils as bu
    bu.upload_artifacts = lambda tmpdir: tmpdir
    if "antenv.axon_hooks" in sys.modules:
        return
    try:
        from trn_agent_boot.trn_boot import _ntff_profile_via_ctypes
        hook = _ntff_profile_via_ctypes("/opt/axon/libaxon_pjrt.so")
        m = types.ModuleType("antenv.axon_hooks")
        m.get_axon_ntff_profile_hook = lambda: hook
        sys.modules["antenv.axon_hooks"] = m
    except Exception:
        pass


def _run(nc, in_maps):
    global LAST_EXEC_NS
    if PROFILE:
        _enable_profiling()
        res = run_bass_kernel_spmd(nc, in_maps, list(range(N_CORES)),
                                   trace=True)
        if res.exec_time_ns is not None:
            LAST_EXEC_NS += res.exec_time_ns
    else:
        res = run_bass_kernel_spmd(nc, in_maps, list(range(N_CORES)))
    return res.results


def _gate_weight_pack(lw, lo):
    """lw [1024, cin+256, 3, 3] -> (wx, wh) each [128, 8*2304]:
    wsb[p, o*2304 + (ti*2+ct)*128 + j] = W[o*128+j, base+ct*128+p, ky, kx]."""
    cin = lw.shape[1] - HID
    out = []
    for base in (0, cin):
        w = np.empty((9, 2, 128, 1024), np.float32)
        for ti, (ky, kx) in enumerate(_taps()):
            for ct in range(2):
                sl = lw[:, base + ct * 128: base + (ct + 1) * 128, ky, kx]
                w[ti, ct] = np.ascontiguousarray(sl.T)
        packed = np.ascontiguousarray(
            w.reshape(9, 2, 128, 8, 128).transpose(2, 3, 0, 1, 4)
        ).reshape(128, 8 * 2304)
        out.append(packed)
    return out[0], out[1]


def kernel(x, w0, b0, g0, be0, m0, v0,
           w1, b1, g1, be1, m1, v1,
           w2, b2, g2, be2, m2, v2,
           lw0, lb0, lw1, lb1):
    global LAST_EXEC_NS
    LAST_EXEC_NS = 0.0
    progs = _get_programs()

    x = np.asarray(x, np.float32)
    nB, nT = x.shape[0], x.shape[1]
    assert (nB, nT) == (B, T)

    # ---- host prep: padded images ----
    imgs = x.reshape(IMGS, H, W)
    xpad = np.zeros((IMGS, XPADN), np.float32)
    v = xpad[:, 1:1 + P0 * P0].reshape(IMGS, P0, P0)
    v[:, 1:129, 1:129] = imgs

    # ---- stem weights with BN folding ----
    def fold(w, b, g, be, m, vv):
        scale = (np.asarray(g) / np.sqrt(np.asarray(vv) + EPS)).astype(np.float32)
        shift = (np.asarray(be) - np.asarray(m) * scale).astype(np.float32)
        bias = np.asarray(b) * scale + shift
        wt = np.asarray(w) * scale[:, None, None, None]
        return wt.astype(np.float32), bias.astype(np.float32)

    w0f, bias0 = fold(w0, b0, g0, be0, m0, v0)
    w1f, bias1 = fold(w1, b1, g1, be1, m1, v1)
    w2f, bias2 = fold(w2, b2, g2, be2, m2, v2)

    w0t = np.zeros((128, 64), np.float32)
    w1t = np.zeros((9, 128, 128), np.float32)
    w2t = np.empty((9, 128, 256), np.float32)
    for ti, (ky, kx) in enumerate(_taps()):
        w0t[ti] = w0f[:, 0, ky, kx]
        w1t[ti, 0:64] = w1f[:, :, ky, kx].T
        w2t[ti] = w2f[:, :, ky, kx].T
    b0a = bias0.reshape(64, 1)
    b1a = bias1.reshape(128, 1)
    b2a = np.ascontiguousarray(bias2.reshape(2, 128).T)

    wx0, wh0 = _gate_weight_pack(np.asarray(lw0, np.float32), lb0)
    wx1, wh1 = _gate_weight_pack(np.asarray(lw1, np.float32), lb1)
    lb0a = np.ascontiguousarray(np.asarray(lb0, np.float32).reshape(8, 128).T)
    lb1a = np.ascontiguousarray(np.asarray(lb1, np.float32).reshape(8, 128).T)

    # ---- launch A ----
    in_a = [{"xpad": xpad[c * IPC:(c + 1) * IPC], "w0t": w0t, "b0t": b0a,
             "w1t": w1t, "b1t": b1a, "w2t": w2t, "b2t": b2a,
             "wx": wx0, "lb": lb0a} for c in range(N_CORES)]
    res = _run(progs["A"], in_a)
    zx0 = np.concatenate([res[c]["zx"] for c in range(N_CORES)], axis=0)
    # zx0: [32, 8, 128, SL], img index = b*T + t

    # ---- launch R (layer 0) ----
    zxr0 = [np.ascontiguousarray(
        zx0[b_ * T:(b_ + 1) * T].transpose(0, 2, 1, 3)).reshape(T, 128, 8 * SL)
        for b_ in range(B)]
    in_r = [{"zx": zxr0[c % B], "wh": wh0} for c in range(N_CORES)]
    res = _run(progs["R"], in_r)
    hpad0 = [res[b]["hpad_seq"] for b in range(B)]  # [T, 2, 128, QL] each

    # ---- launch C (Zx1) ----
    hp_items = np.ascontiguousarray(
        np.concatenate(hpad0, axis=0).transpose(0, 2, 1, 3)
    ).reshape(IMGS, 128, 2 * QL)
    in_c = [{"hpad": hp_items[c * IPC:(c + 1) * IPC], "wx": wx1, "lb": lb1a}
            for c in range(N_CORES)]
    res = _run(progs["C"], in_c)
    zx1 = np.concatenate([res[c]["zx"] for c in range(N_CORES)], axis=0)

    # ---- launch R (layer 1) ----
    zxr1 = [np.ascontiguousarray(
        zx1[b_ * T:(b_ + 1) * T].transpose(0, 2, 1, 3)).reshape(T, 128, 8 * SL)
        for b_ in range(B)]
    in_r = [{"zx": zxr1[c % B], "wh": wh1} for c in range(N_CORES)]
    res = _run(progs["R"], in_r)

    z_seq = np.empty((B, T, HID, 16, 16), np.float32)
    for b_ in range(B):
        hv = res[b_]["hval_seq"]  # [T, 2, 128, 256]
        z_seq[b_] = hv.reshape(T, HID, 16, 16)
    z_last = np.ascontiguousarray(z_seq[:, -1])
    return z_seq, z_last


# revision 20
# speedup vs baseline: 1.5171x; 1.0035x over previous
"""Trainium2 Bass kernel for nn_Encoder (CNN stem + 2-layer ConvLSTM).

Self-contained: builds three Bass programs (compiled once per process) and
orchestrates four SPMD launches on 8 NeuronCores:

  A: CNN stem (3x conv+BN+ReLU+pool) + layer-0 input-gate conv Zx0,
     data-parallel over the 32 (b,t) images (4 per core).
  R: ConvLSTM recurrence (16 sequential steps; hidden-half gate conv +
     gate nonlinearities + state update), data-parallel over batch (B=2).
     Compiled once, launched twice (layer 0 and layer 1).
  C: layer-1 input-gate conv Zx1 over layer-0 hidden states,
     data-parallel over the 32 (b,t) items.

All matmuls run as float32r (TF32-like, ~1 PE cycle/row at free>=256).
Conv3x3 is 9 shifted matmuls accumulated in PSUM over zero-padded
flat buffers with a 1-element lead/tail pad; the pad columns of each
conv output span carry junk that is never read by valid outputs.
"""

import numpy as np

import concourse.bass as bass
import concourse.mybir as mybir
from concourse import bacc
from concourse.bass_utils import run_bass_kernel_spmd
from concourse.tile import TileContext

F32 = mybir.dt.float32
F32R = mybir.dt.float32r
AF = mybir.ActivationFunctionType
ALU = mybir.AluOpType

N_CORES = 8
B, T, H, W = 2, 16, 128, 128
HID = 256
EPS = 1e-5
IMGS = B * T           # 32
IPC = IMGS // N_CORES  # 4 images per core

# stem geometry: (rows, padded width, span length, lead-padded buffer len)
# level 0 input: 128x128 -> padded 130x130
P0 = 130
XPADN = P0 * P0 + 2          # host-side lead/tail padded flat image
# level 1 input: 64x64 -> padded 66
P1, S1 = 66, 64 * 66         # span covers 64 rows x 66 cols
Q1 = 66 * 66 + 2
# level 2 input: 32x32 -> padded 34
P2, S2 = 34, 32 * 34
Q2 = 34 * 34 + 2
# lstm spatial: 16x16 -> padded 18
PL, SL = 18, 16 * 18         # SL = 288
QL = 18 * 18 + 2             # 326

PROFILE = False
LAST_EXEC_NS = 0.0

_PROGRAMS = {}


def _taps():
    return [(ky, kx) for ky in range(3) for kx in range(3)]


def _ap(handle, offset, dims):
    return bass.AP(handle, offset, [list(d) for d in dims])


# --------------------------------------------------------------------------
# shared emitters
# --------------------------------------------------------------------------

def _gate_lhs(wsb, ot, ti, ct, n_ct=2):
    base = ot * 9 * n_ct * 128 + (ti * n_ct + ct) * 128
    return wsb[:, base:base + 128]


def _emit_gate_conv(nc, psum_pool, wsb, src_slices, out_cb, n_ct=2,
                    order=tuple(range(8))):
    """z[ot] = sum_{tap,ct} W[tap,ct,:,ot*128:+128].T @ src[ct][tapoff:+288]."""
    taps = _taps()
    for ot in order:
        ps = psum_pool.tile([128, SL], F32, tag="psg")
        n_k = len(taps) * n_ct
        k = 0
        for ti, (ky, kx) in enumerate(taps):
            for ct in range(n_ct):
                nc.tensor.matmul(ps[:, :], _gate_lhs(wsb, ot, ti, ct, n_ct),
                                 src_slices(ct, ky * PL + kx),
                                 start=(k == 0), stop=(k == n_k - 1))
                k += 1
        out_cb(ot, ps)


def _load_gate_weights(nc, sb, w_dram, n_ct=2, order=tuple(range(8))):
    """DMA host-prepacked [128, 8*9*n_ct*128] f32r weights (otile-major,
    contiguous slabs) into SBUF. First-used slab goes on the scalar HWDGE
    queue (fast, uncontended); the rest stream on SWDGE in use-order."""
    slab = 9 * n_ct * 128
    wsb = sb.tile([128, 8 * slab], F32R, tag="wgate")
    for j, o in enumerate(order):
        eng = nc.scalar if j == 0 else nc.gpsimd
        eng.dma_start(wsb[:, o * slab:(o + 1) * slab],
                      w_dram[:, o * slab:(o + 1) * slab])
    return wsb


# --------------------------------------------------------------------------
# Launch A: CNN stem + Zx0
# --------------------------------------------------------------------------

def build_A():
    nc = bacc.Bacc("TRN2", target_bir_lowering=False, debug=False,
                   num_devices=N_CORES)
    xpad = nc.dram_tensor("xpad", [IPC, XPADN], F32R, kind="ExternalInput")
    w0t = nc.dram_tensor("w0t", [128, 64], F32R, kind="ExternalInput")
    b0t = nc.dram_tensor("b0t", [64, 1], F32, kind="ExternalInput")
    w1t = nc.dram_tensor("w1t", [9, 128, 128], F32R, kind="ExternalInput")
    b1t = nc.dram_tensor("b1t", [128, 1], F32, kind="ExternalInput")
    w2t = nc.dram_tensor("w2t", [9, 128, 256], F32R, kind="ExternalInput")
    b2t = nc.dram_tensor("b2t", [128, 2], F32, kind="ExternalInput")
    wx = nc.dram_tensor("wx", [128, 8 * 2304], F32R, kind="ExternalInput")
    lb = nc.dram_tensor("lb", [128, 8], F32, kind="ExternalInput")
    zx = nc.dram_tensor("zx", [IPC, 8, 128, SL], F32, kind="ExternalOutput")

    taps = _taps()
    with TileContext(nc) as tc:
        with (
            tc.tile_pool(name="wpool", bufs=1) as wp,
            tc.tile_pool(name="pads", bufs=1) as padp,
            tc.tile_pool(name="work", bufs=1) as wk,
            tc.tile_pool(name="psum", bufs=3, space="PSUM") as pp,
            tc.tile_pool(name="psumg", bufs=5, space="PSUM") as ppg,
        ):
            w0sb = wp.tile([128, 64], F32R)
            nc.sync.dma_start(w0sb[:], w0t[:, :])
            w1sb = wp.tile([128, 9 * 128], F32R)
            nc.gpsimd.dma_start(
                w1sb[:].rearrange("p (t o) -> p t o", t=9),
                _ap(w1t, 0, [[128, 128], [128 * 128, 9], [1, 128]]))
            w2sb = wp.tile([128, 9 * 256], F32R)
            nc.gpsimd.dma_start(
                w2sb[:].rearrange("p (t o) -> p t o", t=9),
                _ap(w2t, 0, [[256, 128], [128 * 256, 9], [1, 256]]))
            b0sb = wp.tile([64, 1], F32)
            nc.sync.dma_start(b0sb[:], b0t[:, :])
            b1sb = wp.tile([128, 1], F32)
            nc.sync.dma_start(b1sb[:], b1t[:, :])
            b2sb = wp.tile([128, 2], F32)
            nc.sync.dma_start(b2sb[:], b2t[:, :])
            lbsb = wp.tile([128, 8], F32)
            nc.sync.dma_start(lbsb[:], lb[:, :])
            wxsb = _load_gate_weights(nc, wp, wx)

            x1p = padp.tile([128, Q1], F32R)
            x2p = padp.tile([128, Q2], F32R)
            spd = [[padp.tile([128, QL], F32R, tag=f"spd{o}{par}",
                              name=f"spd{o}{par}") for o in range(2)]
                   for par in range(2)]
            zsb = padp.tile([128, Q1], F32)
            nc.vector.memset(zsb[:], 0.0)
            nc.vector.tensor_copy(x1p[:], zsb[:, 0:Q1])
            nc.scalar.copy(x2p[:], zsb[:, 0:Q2])
            for par in range(2):
                for o in range(2):
                    nc.scalar.copy(spd[par][o][:], zsb[:, 0:QL])

            CH0 = 16   # b0 output rows per chunk
            CH1 = 32   # b1 output rows per chunk
            pch = [wk.tile([128, CH0 * P0], F32R, tag=f"patch{j}",
                           name=f"patch{j}") for j in range(2)]
            nc.vector.tensor_copy(pch[0][:, :], zsb[:, 0:CH0 * P0])
            nc.vector.tensor_copy(pch[1][:, :], zsb[:, 0:CH0 * P0])

            taps_l = taps

            def emit_zx_otile(i, ot):
                ps = ppg.tile([128, SL], F32, tag="psg", name="psg")
                k = 0
                for ti in range(9):
                    ky, kx = taps_l[ti]
                    for ct in range(2):
                        nc.tensor.matmul(
                            ps[:, :], _gate_lhs(wxsb, ot, ti, ct),
                            spd[i % 2][ct][:, ky * PL + kx:
                                           ky * PL + kx + SL],
                            start=(k == 0), stop=(k == 17))
                        k += 1
                zs = wk.tile([128, SL], F32, tag="zxs", bufs=2, name="zxs")
                nc.scalar.activation(zs[:], ps[:, :], AF.Identity,
                                     bias=lbsb[:, ot:ot + 1])
                nc.sync.dma_start(zx[i, ot], zs[:])

            for i in range(IPC):
                # ---- b0: conv 1->64 via im2col (contract 9 pad 128);
                # interleave previous image's Zx0 otiles to keep PE dense --
                for c in range(128 // CH0):
                    if i > 0:
                        emit_zx_otile(i - 1, c)
                    else:
                        # no previous image to interleave: keep the PE busy
                        # through b0's DMA/evict stalls so HAM stays warm
                        for d in range(8):
                            pd = ppg.tile([128, SL], F32, tag="psg",
                                          name="psg")
                            nc.tensor.matmul(pd[0:64, :], w0sb[:, 0:64],
                                             pch[0][:, 0:SL],
                                             start=True, stop=True)
                    patch = pch[c % 2]
                    # patch[3ky+kx, s] = xpad[i][(130ky+kx) + r0*130 + s]
                    for ky in range(3):
                        psrc = _ap(xpad, i * XPADN + c * CH0 * P0 + P0 * ky,
                                   [[1, 3], [1, CH0 * P0]])
                        nc.sync.dma_start(patch[3 * ky:3 * ky + 3, :], psrc)
                    y0 = wk.tile([64, CH0 * P0], F32, tag="y0")
                    nsub = (CH0 * P0 + 511) // 512
                    for s in range(nsub):
                        lo = s * 512
                        ln = min(512, CH0 * P0 - lo)
                        ps = pp.tile([128, 512], F32, tag="pss")
                        nc.tensor.matmul(ps[0:64, :ln], w0sb[:],
                                         patch[:, lo:lo + ln],
                                         start=True, stop=True)
                        if s % 2 == 0:
                            nc.scalar.activation(y0[:, lo:lo + ln],
                                                 ps[0:64, :ln],
                                                 AF.Relu, bias=b0sb[:, 0:1])
                        else:
                            nc.vector.scalar_tensor_tensor(
                                y0[:, lo:lo + ln], ps[0:64, :ln],
                                b0sb[:, 0:1], zsb[0:64, lo:lo + ln],
                                ALU.add, ALU.max)
                    # pool 2x2: span rows CH0 x 130, valid cols 1..128
                    y3 = y0[:].rearrange("p (r c) -> p r c", c=P0)
                    pa = wk.tile([64, CH0 * 64], F32, tag="pa")
                    pa3 = pa[:].rearrange("p (r c) -> p r c", c=64)
                    nc.vector.tensor_tensor(
                        pa3, y3[:, :, 1:129:2], y3[:, :, 2:130:2], ALU.max)
                    r0 = c * CH0 // 2
                    dst = _ap(x1p.tensor, x1p.offset + 68 + r0 * P1,
                              [[x1p.ap[0][0], 64], [P1, CH0 // 2],
                               [1, 64]])
                    nc.vector.tensor_tensor(
                        dst, pa3[:, 0:CH0:2, :], pa3[:, 1:CH0:2, :], ALU.max)

                # ---- b1: conv 64->128 + pool ----
                for c in range(64 // CH1):
                    base = c * CH1 * P1
                    y1 = wk.tile([128, CH1 * P1], F32, tag="y1", bufs=2)
                    nsub = (CH1 * P1 + 511) // 512
                    for s in range(nsub):
                        lo = s * 512
                        ln = min(512, CH1 * P1 - lo)
                        ps = pp.tile([128, 512], F32, tag="pss")
                        for ti, (ky, kx) in enumerate(taps):
                            off = ky * P1 + kx + base + lo
                            nc.tensor.matmul(
                                ps[:, :ln],
                                w1sb[:, ti * 128:(ti + 1) * 128],
                                x1p[:, off:off + ln],
                                start=(ti == 0), stop=(ti == 8))
                        nc.scalar.activation(y1[:, lo:lo + ln], ps[:, :ln],
                                             AF.Relu, bias=b1sb[:, 0:1])
                    y3 = y1[:].rearrange("p (r c) -> p r c", c=P1)
                    pa1 = wk.tile([128, CH1 * 32], F32, tag="pa1")
                    pa3 = pa1[:].rearrange("p (r c) -> p r c", c=32)
                    nc.vector.tensor_tensor(
                        pa3, y3[:, :, 1:65:2], y3[:, :, 2:66:2], ALU.max)
                    r0 = c * CH1 // 2
                    dst = _ap(x2p.tensor, x2p.offset + 36 + r0 * P2,
                              [[x2p.ap[0][0], 128], [P2, CH1 // 2],
                               [1, 32]])
                    nc.vector.tensor_tensor(
                        dst, pa3[:, 0:CH1:2, :], pa3[:, 1:CH1:2, :], ALU.max)

                # ---- b2: conv 128->256 + pool ----
                for o in range(2):
                    y2 = wk.tile([128, S2], F32, tag="y2")
                    nsub = (S2 + 511) // 512
                    for s in range(nsub):
                        lo = s * 512
                        ln = min(512, S2 - lo)
                        ps = pp.tile([128, 512], F32, tag="pss")
                        for ti, (ky, kx) in enumerate(taps):
                            off = ky * P2 + kx + lo
                            nc.tensor.matmul(
                                ps[:, :ln],
                                w2sb[:, ti * 256 + o * 128:
                                     ti * 256 + o * 128 + 128],
                                x2p[:, off:off + ln],
                                start=(ti == 0), stop=(ti == 8))
                        nc.scalar.activation(y2[:, lo:lo + ln], ps[:, :ln],
                                             AF.Relu, bias=b2sb[:, o:o + 1])
                    y3 = y2[:].rearrange("p (r c) -> p r c", c=P2)
                    pa2 = wk.tile([128, 32 * 16], F32, tag="pa2")
                    pa3 = pa2[:].rearrange("p (r c) -> p r c", c=16)
                    nc.vector.tensor_tensor(
                        pa3, y3[:, :, 1:33:2], y3[:, :, 2:34:2], ALU.max)
                    sp = spd[i % 2][o]
                    dst = _ap(sp.tensor, sp.offset + 20,
                              [[sp.ap[0][0], 128], [PL, 16], [1, 16]])
                    nc.vector.tensor_tensor(
                        dst, pa3[:, 0:32:2, :], pa3[:, 1:32:2, :], ALU.max)

            # ---- Zx0 for the last image ----
            for ot in range(8):
                emit_zx_otile(IPC - 1, ot)
    nc.compile()
    return nc


# --------------------------------------------------------------------------
# Launch R: ConvLSTM recurrence (one layer, one batch element per core)
# --------------------------------------------------------------------------

def build_R():
    nc = bacc.Bacc("TRN2", target_bir_lowering=False, debug=False,
                   num_devices=N_CORES)
    zx = nc.dram_tensor("zx", [T, 128, 8 * SL], F32, kind="ExternalInput")
    wh = nc.dram_tensor("wh", [128, 8 * 2304], F32R, kind="ExternalInput")
    hpad_seq = nc.dram_tensor("hpad_seq", [T, 2, 128, QL], F32R,
                              kind="ExternalOutput")
    hval_seq = nc.dram_tensor("hval_seq", [T, 2, 128, 256], F32,
                              kind="ExternalOutput")

    # gate channel blocks: i: ot 0-1, f: 2-3, o: 4-5, g: 6-7
    with TileContext(nc) as tc:
        with (
            tc.tile_pool(name="wpool", bufs=1) as wp,
            tc.tile_pool(name="state", bufs=1) as stp,
            tc.tile_pool(name="work", bufs=2) as wk,
            tc.tile_pool(name="psum", bufs=6, space="PSUM") as pp,
        ):
            whsb = _load_gate_weights(nc, wp, wh,
                                      order=(6, 7, 2, 3, 0, 1, 4, 5))
            hp = [stp.tile([128, QL], F32R, tag=f"hp{c}", name=f"hp{c}")
                  for c in range(2)]
            cs = [stp.tile([128, SL], F32, tag=f"cs{c}", name=f"cs{c}")
                  for c in range(2)]
            zsb = stp.tile([128, QL], F32)
            nc.vector.memset(zsb[:], 0.0)
            for c in range(2):
                nc.vector.tensor_copy(hp[c][:], zsb[:, 0:QL])
                nc.vector.memset(cs[c][:], 0.0)

            taps = _taps()
            for t in range(T):
                zxsb = wk.tile([128, 8 * SL], F32, tag="zxsb")
                nc.sync.dma_start(zxsb[:], zx[t])

                # otile order: g gates (6,7) first, then f (2,3), i (0,1),
                # o last (4,5) -- lets the c-update run under the remaining
                # convs so the serial tail is only sigma(o), h, and the copy.
                ga = {}
                hss = {}
                for ot in (6, 7, 2, 3, 0, 1, 4, 5):
                    ps = pp.tile([128, SL], F32, tag="psr")
                    k = 0
                    for ti, (ky, kx) in enumerate(taps):
                        for ct in range(2):
                            toff = ky * PL + kx
                            nc.tensor.matmul(ps[:, :],
                                             _gate_lhs(whsb, ot, ti, ct),
                                             hp[ct][:, toff:toff + SL],
                                             start=(k == 0), stop=(k == 17))
                            k += 1
                    zt = wk.tile([128, SL], F32, tag=f"zt{ot}", name=f"zt{ot}")
                    nc.vector.tensor_tensor(zt[:], ps[:, :],
                                            zxsb[:, ot * SL:(ot + 1) * SL],
                                            ALU.add)
                    g = wk.tile([128, SL], F32, tag=f"ga{ot}", name=f"ga{ot}")
                    nc.scalar.activation(
                        g[:], zt[:], AF.Tanh if ot >= 6 else AF.Sigmoid)
                    ga[ot] = g

                    if ot in (0, 1):
                        # i gate done; g, f already done: c = f*c + i*g
                        ht = ot
                        t1 = wk.tile([128, SL], F32, tag=f"t1{ht}",
                                     name=f"t1{ht}")
                        nc.vector.tensor_tensor(t1[:], ga[2 + ht][:],
                                                cs[ht][:], ALU.mult)
                        t2 = wk.tile([128, SL], F32, tag=f"t2{ht}",
                                     name=f"t2{ht}")
                        nc.vector.tensor_tensor(t2[:], ga[ht][:],
                                                ga[6 + ht][:], ALU.mult)
                        nc.vector.tensor_tensor(cs[ht][:], t1[:], t2[:],
                                                ALU.add)
                        th = wk.tile([128, SL], F32, tag=f"th{ht}",
                                     name=f"th{ht}")
                        nc.scalar.activation(th[:], cs[ht][:], AF.Tanh)
                        hss[ht] = th
                    if ot in (4, 5):
                        ht = ot - 4
                        hs = wk.tile([128, SL], F32, tag=f"hs{ht}",
                                     name=f"hs{ht}")
                        nc.vector.tensor_tensor(hs[:], ga[ot][:],
                                                hss[ht][:], ALU.mult)
                        hss[ht] = hs
                # hp may only be overwritten after ALL 8 otile convs of this
                # step have read h_{t-1}; emit the copies after the loop.
                for ht in range(2):
                    hs = hss[ht]
                    hpt = hp[ht]
                    dst = _ap(hpt.tensor, hpt.offset + 20,
                              [[hpt.ap[0][0], 128], [PL, 16], [1, 16]])
                    src = _ap(hs.tensor, hs.offset + 1,
                              [[hs.ap[0][0], 128], [PL, 16], [1, 16]])
                    nc.vector.tensor_copy(dst, src)
                    nc.sync.dma_start(hpad_seq[t, ht], hpt[:])
                    nc.sync.dma_start(hval_seq[t, ht], src)
    nc.compile()
    return nc


# --------------------------------------------------------------------------
# Launch C: layer-1 input-gate conv over layer-0 hidden states
# --------------------------------------------------------------------------

def build_C():
    nc = bacc.Bacc("TRN2", target_bir_lowering=False, debug=False,
                   num_devices=N_CORES)
    hpad = nc.dram_tensor("hpad", [IPC, 128, 2 * QL], F32R,
                          kind="ExternalInput")
    wx = nc.dram_tensor("wx", [128, 8 * 2304], F32R, kind="ExternalInput")
    lb = nc.dram_tensor("lb", [128, 8], F32, kind="ExternalInput")
    zx = nc.dram_tensor("zx", [IPC, 8, 128, SL], F32, kind="ExternalOutput")

    with TileContext(nc) as tc:
        with (
            tc.tile_pool(name="wpool", bufs=1) as wp,
            tc.tile_pool(name="work", bufs=2) as wk,
            tc.tile_pool(name="psum", bufs=4, space="PSUM") as pp,
        ):
            wxsb = _load_gate_weights(nc, wp, wx)
            lbsb = wp.tile([128, 8], F32)
            nc.sync.dma_start(lbsb[:], lb[:, :])
            for i in range(IPC):
                hsb = wk.tile([128, 2 * QL], F32R, tag="hsb")
                nc.sync.dma_start(hsb[:], hpad[i])

                def out_cb(ot, ps, i=i):
                    zs = wk.tile([128, SL], F32, tag="zxs")
                    nc.scalar.activation(zs[:], ps[:, :], AF.Identity,
                                         bias=lbsb[:, ot:ot + 1])
                    nc.sync.dma_start(zx[i, ot], zs[:])

                _emit_gate_conv(
                    nc, pp, wxsb,
                    lambda ct, toff, hsb=hsb: hsb[:, ct * QL + toff:
                                                  ct * QL + toff + SL],
                    out_cb)
    nc.compile()
    return nc


# --------------------------------------------------------------------------
# host orchestration
# --------------------------------------------------------------------------

def _get_programs():
    if not _PROGRAMS:
        _PROGRAMS["A"] = build_A()
        _PROGRAMS["R"] = build_R()
        _PROGRAMS["C"] = build_C()
    return _PROGRAMS


def _enable_profiling():
    import sys
    import types
    import concourse.bass_utils as bu
    bu.upload_artifacts = lambda tmpdir: tmpdir
    if "antenv.axon_hooks" in sys.modules:
        return
    try:
        from trn_agent_boot.trn_boot import _ntff_profile_via_ctypes
        hook = _ntff_profile_via_ctypes("/opt/axon/libaxon_pjrt.so")
        m = types.ModuleType("antenv.axon_hooks")
        m.get_axon_ntff_profile_hook = lambda: hook
        sys.modules["antenv.axon_hooks"] = m
    except Exception:
        pass


def _run(nc, in_maps):
    global LAST_EXEC_NS
    if PROFILE:
        _enable_profiling()
        res = run_bass_kernel_spmd(nc, in_maps, list(range(N_CORES)),
                                   trace=True)
        if res.exec_time_ns is not None:
            LAST_EXEC_NS += res.exec_time_ns
    else:
        res = run_bass_kernel_spmd(nc, in_maps, list(range(N_CORES)))
    return res.results


def _gate_weight_pack(lw, lo):
    """lw [1024, cin+256, 3, 3] -> (wx, wh) each [128, 8*2304]:
    wsb[p, o*2304 + (ti*2+ct)*128 + j] = W[o*128+j, base+ct*128+p, ky, kx]."""
    cin = lw.shape[1] - HID
    out = []
    for base in (0, cin):
        w = np.empty((9, 2, 128, 1024), np.float32)
        for ti, (ky, kx) in enumerate(_taps()):
            for ct in range(2):
                sl = lw[:, base + ct * 128: base + (ct + 1) * 128, ky, kx]
                w[ti, ct] = np.ascontiguousarray(sl.T)
        packed = np.ascontiguousarray(
            w.reshape(9, 2, 128, 8, 128).transpose(2, 3, 0, 1, 4)
        ).reshape(128, 8 * 2304)
        out.append(packed)
    return out[0], out[1]


def kernel(x, w0, b0, g0, be0, m0, v0,
           w1, b1, g1, be1, m1, v1,
           w2, b2, g2, be2, m2, v2,
           lw0, lb0, lw1, lb1):
    global LAST_EXEC_NS
    LAST_EXEC_NS = 0.0
    progs = _get_programs()

    x = np.asarray(x, np.float32)
    nB, nT = x.shape[0], x.shape[1]
    assert (nB, nT) == (B, T)

    # ---- host prep: padded images ----
    imgs = x.reshape(IMGS, H, W)
    xpad = np.zeros((IMGS, XPADN), np.float32)
    v = xpad[:, 1:1 + P0 * P0].reshape(IMGS, P0, P0)
    v[:, 1:129, 1:129] = imgs

    # ---- stem weights with BN folding ----
    def fold(w, b, g, be, m, vv):
        scale = (np.asarray(g) / np.sqrt(np.asarray(vv) + EPS)).astype(np.float32)
        shift = (np.asarray(be) - np.asarray(m) * scale).astype(np.float32)
        bias = np.asarray(b) * scale + shift
        wt = np.asarray(w) * scale[:, None, None, None]
        return wt.astype(np.float32), bias.astype(np.float32)

    w0f, bias0 = fold(w0, b0, g0, be0, m0, v0)
    w1f, bias1 = fold(w1, b1, g1, be1, m1, v1)
    w2f, bias2 = fold(w2, b2, g2, be2, m2, v2)

    w0t = np.zeros((128, 64), np.float32)
    w1t = np.zeros((9, 128, 128), np.float32)
    w2t = np.empty((9, 128, 256), np.float32)
    for ti, (ky, kx) in enumerate(_taps()):
        w0t[ti] = w0f[:, 0, ky, kx]
        w1t[ti, 0:64] = w1f[:, :, ky, kx].T
        w2t[ti] = w2f[:, :, ky, kx].T
    b0a = bias0.reshape(64, 1)
    b1a = bias1.reshape(128, 1)
    b2a = np.ascontiguousarray(bias2.reshape(2, 128).T)

    wx0, wh0 = _gate_weight_pack(np.asarray(lw0, np.float32), lb0)
    wx1, wh1 = _gate_weight_pack(np.asarray(lw1, np.float32), lb1)
    lb0a = np.ascontiguousarray(np.asarray(lb0, np.float32).reshape(8, 128).T)
    lb1a = np.ascontiguousarray(np.asarray(lb1, np.float32).reshape(8, 128).T)

    # ---- launch A ----
    in_a = [{"xpad": xpad[c * IPC:(c + 1) * IPC], "w0t": w0t, "b0t": b0a,
             "w1t": w1t, "b1t": b1a, "w2t": w2t, "b2t": b2a,
             "wx": wx0, "lb": lb0a} for c in range(N_CORES)]
    res = _run(progs["A"], in_a)
    zx0 = np.concatenate([res[c]["zx"] for c in range(N_CORES)], axis=0)
    # zx0: [32, 8, 128, SL], img index = b*T + t

    # ---- launch R (layer 0) ----
    zxr0 = [np.ascontiguousarray(
        zx0[b_ * T:(b_ + 1) * T].transpose(0, 2, 1, 3)).reshape(T, 128, 8 * SL)
        for b_ in range(B)]
    in_r = [{"zx": zxr0[c % B], "wh": wh0} for c in range(N_CORES)]
    res = _run(progs["R"], in_r)
    hpad0 = [res[b]["hpad_seq"] for b in range(B)]  # [T, 2, 128, QL] each

    # ---- launch C (Zx1) ----
    hp_items = np.ascontiguousarray(
        np.concatenate(hpad0, axis=0).transpose(0, 2, 1, 3)
    ).reshape(IMGS, 128, 2 * QL)
    in_c = [{"hpad": hp_items[c * IPC:(c + 1) * IPC], "wx": wx1, "lb": lb1a}
            for c in range(N_CORES)]
    res = _run(progs["C"], in_c)
    zx1 = np.concatenate([res[c]["zx"] for c in range(N_CORES)], axis=0)

    # ---- launch R (layer 1) ----
    zxr1 = [np.ascontiguousarray(
        zx1[b_ * T:(b_ + 1) * T].transpose(0, 2, 1, 3)).reshape(T, 128, 8 * SL)
        for b_ in range(B)]
    in_r = [{"zx": zxr1[c % B], "wh": wh1} for c in range(N_CORES)]
    res = _run(progs["R"], in_r)

    z_seq = np.empty((B, T, HID, 16, 16), np.float32)
    for b_ in range(B):
        hv = res[b_]["hval_seq"]  # [T, 2, 128, 256]
        z_seq[b_] = hv.reshape(T, HID, 16, 16)
    z_last = np.ascontiguousarray(z_seq[:, -1])
    return z_seq, z_last
